# revision 20
# baseline (speedup 1.0000x reference)
"""Trainium2 Bass kernel for the 4-kernel MMD permutation test (nn_DUAL_78237124264373).

Math (per core, 25 of the 200 permutations; everything else replicated):
  Z = [X; Y] (768 x 64). The full squared-distance matrix lands in PSUM as a
  single rank-66 matmul d2 = L^T R with L = [Zt; 1; sq], R = [-2 Zt; sq; 1]
  (sq folded in on the host), so the gaussian kernels exp straight out of
  PSUM with per-kernel scalar scale/bias only. The laplacian kernels go
  through a DVE clamp -> one wide Scalar sqrt -> exp.
  With a_p the X-half indicator of permutation p, every U_b entry reduces to
     U_b = kap*(q0 - arow0) + W_corr @ e_k + (2/c2)*t + C_k
  where q0 = a K0 a, arow0 = a K0 1 come from M0 = A_aug K0 (col-tiled so
  kernel k / perm i stats live at partition 32k+i), e = the zeroed stripe
  K0[j, 384+j], and t is the per-permutation paired-sample sum. Pair and
  stripe squared distances are host-precomputed (d2p, 40KB) and share the
  device sqrt/exp path, so no pair-row gather DMA is needed. The U column
  falls out of q0/arow at the two augmented indicator rows (25 = all-X,
  26 = all-Y).

Final assembly transposes the per-partition stats onto rows with one PE
transpose matmul (identity rhs) and emits the whole [4, 26] result in one
DMA.
"""

import sys

import numpy as np

if "/opt/trn_rl_repo" not in sys.path:
    sys.path.insert(0, "/opt/trn_rl_repo")

import concourse.bacc as bacc
import concourse.bass as bass
import concourse.mybir as mybir
import concourse.tile as tile
from concourse import bass_utils

N = 384
NM = 768
D = 64
NPER = 200
NC = 8
PPC = NPER // NC  # 25
NP6 = 6 * NM      # 4608
WK = NP6 + 78     # 4686 = kernel-matrix cols + pair/stripe cols
C1 = float(N * (N - 1))
C2 = float(N * N)
KAP = np.float32(2.0 / C1 + 2.0 / C2)
CB1 = np.float32(1.0 / C1 + 2.0 / C2)
CB2 = np.float32(1.0 / C1)
TCO = np.float32(2.0 / C2)
IC1 = np.float32(1.0 / C1)
IC2 = np.float32(1.0 / C2)
KERNELS = ("gaussian", "laplacian", "gaussian", "laplacian")

F32 = mybir.dt.float32
F32R = mybir.dt.float32r
BF16 = mybir.dt.bfloat16
AF = mybir.ActivationFunctionType
ALU = mybir.AluOpType

# bigin f32 column layout
BG_ASTK = 0            # [128, 768] A_aug stacked per kernel group
BG_D2P = 768           # [128, 78] pair + stripe squared distances
BG_IDENT = 846         # [128, 128] identity (PE transpose rhs)
BG_AUX = 974           # [128, 10] per-kernel act scale/bias + eps
BG_AUX4 = 984          # [1, 8] row-0 diag constants
BG_W = 992
# atpb bf16 column layout
AB_ATP = 0             # [128, 192] A_aug^T chunks (32-padded)
AB_WCT = 192           # [128, 96] W_corr^T chunks (row 25 = ones -> se)
AB_W = 288


def _build():
    nc = bacc.Bacc("TRN2", target_bir_lowering=False, debug=False)
    with tile.TileContext(nc) as tc:
        with tc.tile_pool(name="dram", bufs=1, space="DRAM") as dram, \
             tc.tile_pool(name="io", bufs=1) as io, \
             tc.tile_pool(name="big", bufs=1) as big, \
             tc.tile_pool(name="kpool", bufs=4) as kpool, \
             tc.tile_pool(name="scr", bufs=1) as scr, \
             tc.tile_pool(name="sml", bufs=1) as sml:

            lrb_d = dram.tile([70, 2 * NM], BF16, kind="ExternalInput",
                              name="lrb", uniquify=False)
            bigin_d = dram.tile([128, BG_W], F32, kind="ExternalInput",
                                name="bigin", uniquify=False)
            atpb_d = dram.tile([128, AB_W], BF16, kind="ExternalInput",
                               name="atpb", uniquify=False)
            out_d = dram.tile([4, 1 + PPC], F32, kind="ExternalOutput",
                              name="out", uniquify=False)

            # ---- phase 0: input DMAs (L|R first: it gates the PE) ----
            lrb = io.tile([70, 2 * NM], BF16, name="lrb_sb")
            nc.sync.dma_start(out=lrb[:], in_=lrb_d[:])
            bigin = io.tile([128, BG_W], F32, name="bigin_sb")
            nc.sync.dma_start(out=bigin[:, BG_AUX:BG_W],
                              in_=bigin_d[:, BG_AUX:BG_W])
            atpb = io.tile([128, AB_W], BF16, name="atpb_sb")
            nc.sync.dma_start(out=atpb[:], in_=atpb_d[:])
            nc.sync.dma_start(out=bigin[:, 0:BG_AUX],
                              in_=bigin_d[:, 0:BG_AUX])

            astk = bigin[:, BG_ASTK:BG_ASTK + NM]
            d2pv = bigin[:, BG_D2P:BG_D2P + 78]
            identf = bigin[:, BG_IDENT:BG_IDENT + 128]
            aux = bigin[:, BG_AUX:BG_AUX + 10]
            aux4 = bigin[0:1, BG_AUX4:BG_AUX4 + 8]
            atp = atpb[:, AB_ATP:AB_ATP + 192]
            wct = atpb[:, AB_WCT:AB_WCT + 96]

            onesb = io.tile([128, 1], BF16, name="onesb_sb")
            nc.vector.memset(onesb[:], 1.0)
            onesf = io.tile([128, 1], F32, name="onesf_sb")
            nc.vector.memset(onesf[:], 1.0)

            # d2sb cols 0:4608 = clamped d2 row-tiles; 4608:4686 = host pair
            # d2, so ONE wide sqrt covers both.
            d2sb = big.tile([128, WK], F32, name="d2sb")
            dist = big.tile([128, WK], F32, name="dist_sb")
            kts = [kpool.tile([128, WK], BF16, name=f"kt{k}", tag="kt")
                   for k in range(4)]

            # Warm-up: loads the Exp act table while DMAs are in flight.
            warm = sml.tile([128, 1], F32, name="warm")
            nc.scalar.activation(warm[:], onesf[:], AF.Exp, scale=1.0,
                                 bias=onesf[:, 0:1])

            with tc.tile_pool(name="psM", bufs=1, space="PSUM") as psM:
                ps_m = psM.tile([128, NM], F32, name="ps_m")

                def m0(k, c, start, stop):
                    pr = slice(32 * k, 32 * k + 32)
                    lhsA = atp[:, 32 * c:32 * (c + 1)]
                    for s in (slice(0, 512), slice(512, NM)):
                        nc.tensor.matmul(ps_m[pr, s], lhsA,
                                         kts[k][:, NM * c + s.start:
                                                NM * c + s.stop],
                                         start=start, stop=stop,
                                         tile_position=(0, 32 * k),
                                         skip_group_check=True)

                with tc.tile_pool(name="psA", bufs=3, space="PSUM") as psA:
                    # ---- phase 1: d2 row-tiles on the PE (f32r), gaussian
                    # exps straight from PSUM, DVE clamp into d2sb ----
                    for r in range(6):
                        ps_d2 = psA.tile([128, NM], F32, name=f"ps_d2_{r}",
                                         tag="d2")
                        # K=70 bf16: rows 0:64 Zt / -2Zt, rows 64:70 carry
                        # the ones/sq rank-2 terms, sq split hi+mid+lo and
                        # derived from the bf16 z so the diagonal cancels.
                        lhsZ = lrb[:, 128 * r:128 * (r + 1)]
                        for s in (slice(0, 512), slice(512, NM)):
                            nc.tensor.matmul(ps_d2[:, s], lhsZ,
                                             lrb[:, NM + s.start:NM + s.stop],
                                             start=True, stop=True)
                        sl = slice(NM * r, NM * (r + 1))
                        for k in (0, 2):
                            nc.scalar.activation(
                                kts[k][:, sl], ps_d2[:], AF.Exp,
                                scale=aux[:, 2 * k:2 * k + 1],
                                bias=aux[:, 2 * k + 1:2 * k + 2])
                        nc.vector.tensor_scalar(
                            out=d2sb[:, sl], in0=ps_d2[:],
                            scalar1=0.0, scalar2=0.0,
                            op0=ALU.max, op1=ALU.add)
                    # pair d2 into the tail columns, then gaussian pair exps
                    nc.vector.tensor_copy(d2sb[:, NP6:WK], d2pv)
                    for k in (0, 2):
                        nc.scalar.activation(
                            kts[k][:, NP6:WK], d2sb[:, NP6:WK], AF.Exp,
                            scale=aux[:, 2 * k:2 * k + 1],
                            bias=aux[:, 2 * k + 1:2 * k + 2])

                    # ---- phase 2: gaussian M0 accumulation (col-tiled;
                    # kernel k's stats land at partitions 32k+i) ----
                    for c in range(6):
                        for k in (0, 2):
                            m0(k, c, start=(c == 0), stop=(c == 5))

                # ---- phase 3: dist = sqrt(d2 + 1e-12) incl pair cols,
                # laplacian exps (half-tiles, k-interleaved), lap M0 ----
                nc.scalar.activation(dist[:], d2sb[:], AF.Sqrt,
                                     scale=1.0, bias=aux[:, 8:9])

                with tc.tile_pool(name="psS", bufs=1, space="PSUM") as psS:
                    ps_tr = psS.tile([1, 384], F32, name="ps_tr")
                    ps_corr = psS.tile([128, 1], F32, name="ps_corr")
                    ps_T = psS.tile([1, 512], F32, name="ps_T")

                    # stk cols: 0 = U_b body, 1 = kap*q0, 2 = kap*arow,
                    # 3 = corr (copied for the se row)
                    stk = sml.tile([128, 4], F32, name="stk")
                    sB = scr.tile([128, NM], F32, name="sB", tag="sB")
                    sP = scr.tile([128, NM], F32, name="sP", tag="sP")
                    trow = sml.tile([1, 4 * PPC], F32, name="trow")

                    def corr_k(k):
                        for c in range(3):
                            nc.tensor.matmul(
                                ps_corr[32 * k:32 * k + 32, 0:1],
                                wct[:, 32 * c:32 * (c + 1)],
                                kts[k][:, NP6 + 75 + c:NP6 + 76 + c],
                                start=(c == 0), stop=(c == 2),
                                tile_position=(0, 32 * k),
                                skip_group_check=True)

                    def trow_mm(k):
                        nc.tensor.matmul(ps_tr[0:1, 96 * k:96 * k + 75],
                                         onesb[:, 0:1],
                                         kts[k][:, NP6:NP6 + 75],
                                         start=True, stop=True)

                    def trow_red(k):
                        nc.vector.tensor_reduce(
                            trow[0:1, PPC * k:PPC * (k + 1)],
                            ps_tr[0:1, 96 * k:96 * k + 75].rearrange(
                                "o (p t) -> o p t", t=3),
                            axis=mybir.AxisListType.X, op=ALU.add)

                    def stat_k(k, arow_on_act=False):
                        # q0 in one fused DVE op (astk pre-scaled by kap);
                        # arow on DVE, or on the idle Scalar engine for the
                        # last kernel (GPSIMD cannot read PSUM).
                        pr = slice(32 * k, 32 * k + 32)
                        nc.vector.tensor_tensor(out=sB[pr, :],
                                                in0=ps_m[pr, :],
                                                in1=astk[pr, :], op=ALU.mult)
                        nc.vector.tensor_reduce(stk[pr, 1:2], sB[pr, :],
                                                axis=mybir.AxisListType.X,
                                                op=ALU.add)
                        if arow_on_act:
                            nc.scalar.activation(
                                sP[pr, :], ps_m[pr, :], AF.Copy,
                                scale=float(KAP), bias=0.0,
                                accum_out=stk[pr, 2:3])
                        else:
                            nc.vector.tensor_scalar(
                                out=sP[pr, :], in0=ps_m[pr, :],
                                scalar1=float(KAP), scalar2=0.0,
                                op0=ALU.mult, op1=ALU.add,
                                accum_out=stk[pr, 2:3])

                    # gaussian tails (early; PE idles here anyway)
                    for k in (0, 2):
                        trow_mm(k)
                        corr_k(k)
                        stat_k(k)
                        trow_red(k)

                    # laplacian exps in half-tiles so M0 can chase them
                    H = 3 * NM  # 2304
                    for k in (1, 3):
                        nc.scalar.activation(
                            kts[k][:, 0:H], dist[:, 0:H], AF.Exp,
                            scale=aux[:, 2 * k:2 * k + 1],
                            bias=aux[:, 9:10])
                    for k in (1, 3):
                        nc.scalar.activation(
                            kts[k][:, H:WK], dist[:, H:WK], AF.Exp,
                            scale=aux[:, 2 * k:2 * k + 1],
                            bias=aux[:, 9:10])
                    for k in (1, 3):
                        for c in range(3):
                            m0(k, c, start=(c == 0), stop=False)
                    for c in range(3, 6):
                        m0(1, c, start=False, stop=(c == 5))
                    trow_mm(1)
                    corr_k(1)
                    stat_k(1)
                    trow_red(1)
                    for c in range(3, 6):
                        m0(3, c, start=False, stop=(c == 5))
                    trow_mm(3)
                    corr_k(3)
                    stat_k(3, arow_on_act=True)
                    trow_red(3)
                    nc.vector.tensor_scalar_mul(trow[:], trow[:], float(TCO))
                    nc.vector.tensor_copy(stk[:, 3:4], ps_corr[:])

                    # transpose q0/arow/corr rows first, U_b column last
                    for j in (1, 2, 3):
                        nc.tensor.matmul(ps_T[0:1, 128 * j:128 * (j + 1)],
                                         stk[:, j:j + 1], identf,
                                         is_transpose=True,
                                         start=True, stop=True)
                    u0 = sml.tile([128, 1], F32, name="u0")
                    nc.vector.tensor_tensor(out=u0[:], in0=stk[:, 1:2],
                                            in1=stk[:, 2:3], op=ALU.subtract)
                    nc.vector.tensor_tensor(out=stk[:, 0:1], in0=u0[:],
                                            in1=ps_corr[:], op=ALU.add)
                    nc.tensor.matmul(ps_T[0:1, 0:128], stk[:, 0:1], identf,
                                     is_transpose=True, start=True, stop=True)
                    frow = sml.tile([1, 384], F32, name="frow")
                    nc.vector.tensor_copy(frow[:], ps_T[0:1, 128:512])

                    # ---- U stats on transposed rows (all kap-scaled) ----
                    def fr(base, step=32, count=4):
                        ap = frow[0:1, base:base + 1]
                        return bass.AP(ap.tensor, ap.offset,
                                       [ap.ap[0], [step, count]])

                    IC1K = float(IC1 / KAP)
                    XXv = fr(25)         # kap * 1_X K0 1_X
                    YYv = fr(26)         # kap * 1_Y K0 1_Y
                    a25v = fr(128 + 25)  # kap * 1_X K0 1
                    a26v = fr(128 + 26)  # kap * 1_Y K0 1
                    sevv = fr(256 + 25)  # kap * se_k (kap row of W_corr)
                    s12 = sml.tile([1, 4], F32, name="s12")
                    nc.vector.tensor_tensor(out=s12[:], in0=a25v, in1=a26v,
                                            op=ALU.add)
                    nc.vector.tensor_tensor(out=s12[:], in0=s12[:], in1=sevv,
                                            op=ALU.subtract)
                    nc.vector.tensor_tensor(out=s12[:], in0=s12[:],
                                            in1=aux4[0:1, 0:4],
                                            op=ALU.subtract)
                    ck = sml.tile([1, 4], F32, name="ck")
                    nc.vector.tensor_scalar_mul(ck[:], s12[:], IC1K)
                    u1 = sml.tile([1, 4], F32, name="u1")
                    nc.vector.tensor_tensor(out=u1[:], in0=XXv, in1=YYv,
                                            op=ALU.add)
                    nc.vector.tensor_tensor(out=u1[:], in0=u1[:],
                                            in1=aux4[0:1, 0:4],
                                            op=ALU.subtract)
                    nc.vector.tensor_scalar_mul(u1[:], u1[:], IC1K)
                    u2 = sml.tile([1, 4], F32, name="u2")
                    nc.vector.tensor_tensor(out=u2[:], in0=a25v, in1=XXv,
                                            op=ALU.subtract)
                    nc.vector.tensor_tensor(out=u2[:], in0=u2[:], in1=sevv,
                                            op=ALU.subtract)
                    nc.vector.tensor_scalar_mul(u2[:], u2[:],
                                                float(-2.0 * IC2 / KAP))

                    # ---- contiguous [1, 104] result, one DMA ----
                    uball = sml.tile([1, 4 * (1 + PPC)], F32, name="uball")
                    uball0 = uball[0:1, 0:1]
                    uFv = bass.AP(uball0.tensor, uball0.offset,
                                  [uball0.ap[0], [1 + PPC, 4]])
                    nc.vector.tensor_tensor(out=uFv, in0=u1[:], in1=u2[:],
                                            op=ALU.add)
                    ubv = bass.AP(uball0.tensor, uball0.offset + 1,
                                  [uball0.ap[0], [1 + PPC, 4], [1, PPC]])
                    ub_src = ps_T[0:1, 0:128].rearrange(
                        "o (k p) -> o k p", p=32)
                    ckap = ck[0:1, 0:1]
                    ck_b = bass.AP(ckap.tensor, ckap.offset,
                                   [ckap.ap[0], [1, 4], [0, PPC]])
                    nc.vector.tensor_tensor(out=ubv,
                                            in0=ub_src[0:1, :, 0:PPC],
                                            in1=ck_b, op=ALU.add)
                    trow_v = trow[0:1, :].rearrange("o (k p) -> o k p", p=PPC)
                    nc.vector.tensor_tensor(out=ubv, in0=ubv, in1=trow_v,
                                            op=ALU.add)
                    nc.sync.dma_start(
                        out=out_d[:, :],
                        in_=uball[0:1, :].rearrange("o (k w) -> o k w",
                                                    w=1 + PPC))

    nc.compile()
    return nc


def _host_prep(X, Y, bandwidths, perms):
    X = np.ascontiguousarray(X, np.float32)
    Y = np.ascontiguousarray(Y, np.float32)
    perms = np.ascontiguousarray(perms, np.int32)
    import ml_dtypes

    Zf = np.concatenate([X, Y], 0)                  # [768, 64]
    Ztb = Zf.T.astype(ml_dtypes.bfloat16)           # [64, 768] device dtype
    Zb64 = Ztb.astype(np.float64).T                 # bf16-rounded z, f64
    sqb = np.einsum("ij,ij->i", Zb64, Zb64)         # exact device row norms
    Z64 = Zf.astype(np.float64)
    sq64 = np.einsum("ij,ij->i", Z64, Z64)
    sqhi = sqb.astype(np.float32).astype(ml_dtypes.bfloat16)
    r1 = sqb - sqhi.astype(np.float64)
    sqmid = r1.astype(np.float32).astype(ml_dtypes.bfloat16)
    r2 = r1 - sqmid.astype(np.float64)
    sqlo = r2.astype(np.float32).astype(ml_dtypes.bfloat16)
    lrb = np.zeros((70, 2 * NM), ml_dtypes.bfloat16)
    lrb[0:D, 0:NM] = Ztb
    lrb[0:D, NM:] = (-2.0 * Ztb.astype(np.float32)).astype(ml_dtypes.bfloat16)
    lrb[D:D + 3, 0:NM] = 1.0
    lrb[D + 3, 0:NM] = sqhi
    lrb[D + 4, 0:NM] = sqmid
    lrb[D + 5, 0:NM] = sqlo
    lrb[D, NM:] = sqhi
    lrb[D + 1, NM:] = sqmid
    lrb[D + 2, NM:] = sqlo
    lrb[D + 3:D + 6, NM:] = 1.0

    b = np.asarray(bandwidths, np.float64)
    gs = (-1.0 / (b * b)).astype(np.float32)
    ls = (-1.0 / b).astype(np.float32)
    aux = np.zeros((128, 10), np.float32)
    BETA = 0.01
    aux[:, 8] = BETA
    d0c = np.zeros(4, np.float64)
    for k, kern in enumerate(KERNELS):
        if kern == "gaussian":
            aux[:, 2 * k] = gs[k]
            aux[:, 2 * k + 1] = (gs[k].astype(np.float64) * 1e-12
                                 ).astype(np.float32)
            d0c[k] = np.exp(-1e-12 / (b[k] * b[k]))
        else:
            aux[:, 2 * k] = ls[k]
            d0c[k] = np.exp(-np.sqrt(BETA) / b[k])
    aux4 = np.zeros(8, np.float32)
    aux4[0:4] = (768.0 * d0c * float(KAP)).astype(np.float32)

    maps = []
    for cid in range(NC):
        pm = perms[cid * PPC:(cid + 1) * PPC]
        A = np.zeros((27, NM), np.float32)
        A[np.arange(PPC)[:, None], pm[:, :N]] = 1
        A[25, :N] = 1
        A[26, N:] = 1
        astk = np.zeros((128, NM), np.float32)
        for k in range(4):
            astk[32 * k:32 * k + 27] = A * KAP
        atp = np.zeros((128, 6 * 32), np.float32)
        for c in range(6):
            atp[:, 32 * c:32 * c + 27] = A[:, 128 * c:128 * (c + 1)].T
        A1 = A[:PPC, :N]
        A2 = A[:PPC, N:]
        Wc = (-KAP * (A1 * A2) + CB1 * A1 + CB2 * A2).astype(np.float32)
        wct = np.zeros((128, 3 * 32), np.float32)
        for c in range(3):
            wct[:, 32 * c:32 * c + PPC] = Wc[:, 128 * c:128 * (c + 1)].T
            wct[:, 32 * c + 25] = KAP  # kap*se_k extraction row

        # pair + stripe squared distances, f64 on host
        pX = pm[:, :N].astype(np.int64).ravel()
        pY = pm[:, N:].astype(np.int64).ravel()
        j = np.arange(N)
        pa = np.concatenate([pX, j])
        pb = np.concatenate([pY, N + j])
        d2pair = (sq64[pa] + sq64[pb]
                  - 2.0 * np.einsum("ij,ij->i", Z64[pa], Z64[pb]))
        d2pair = np.maximum(d2pair, 0.0) + 1e-12
        stripe = pY == pX + N
        d2pair[:N * PPC][stripe] = 1e12  # zeroed-stripe pairs: f_k -> 0
        d2p = d2pair.astype(np.float32).reshape(78, 128).T  # [128, 78]

        bigin = np.zeros((128, BG_W), np.float32)
        bigin[:, BG_ASTK:BG_ASTK + NM] = astk
        bigin[:, BG_D2P:BG_D2P + 78] = d2p
        bigin[:, BG_IDENT:BG_IDENT + 128] = np.eye(128, dtype=np.float32)
        bigin[:, BG_AUX:BG_AUX + 10] = aux
        bigin[0, BG_AUX4:BG_AUX4 + 8] = aux4
        atpb = np.zeros((128, AB_W), np.float32)
        atpb[:, AB_ATP:AB_ATP + 192] = atp
        atpb[:, AB_WCT:AB_WCT + 96] = wct
        maps.append(dict(lrb=lrb, bigin=bigin,
                         atpb=atpb.astype(ml_dtypes.bfloat16)))
    return maps


_NC_CACHE = None


def _get_nc():
    global _NC_CACHE
    if _NC_CACHE is None:
        _NC_CACHE = _build()
    return _NC_CACHE


def kernel(X, Y, bandwidths, perms):
    nc = _get_nc()
    in_maps = _host_prep(X, Y, bandwidths, perms)
    res = bass_utils.run_bass_kernel_spmd(nc, in_maps, list(range(NC)))
    full = np.zeros((4, 1 + NPER), np.float32)
    full[:, 0] = res.results[0]["out"][:, 0]
    for cid in range(NC):
        full[:, 1 + cid * PPC:1 + (cid + 1) * PPC] = \
            res.results[cid]["out"][:, 1:]
    return full


# revision 22
# speedup vs baseline: 1.0467x; 1.0467x over previous
"""Trainium2 Bass kernel for the 4-kernel MMD permutation test (nn_DUAL_78237124264373).

Math (per core, 25 of the 200 permutations; everything else replicated):
  Z = [X; Y] (768 x 64). The full squared-distance matrix lands in PSUM as a
  single rank-66 matmul d2 = L^T R with L = [Zt; 1; sq], R = [-2 Zt; sq; 1]
  (sq folded in on the host), so the gaussian kernels exp straight out of
  PSUM with per-kernel scalar scale/bias only. The laplacian kernels go
  through a DVE clamp -> one wide Scalar sqrt -> exp.
  With a_p the X-half indicator of permutation p, every U_b entry reduces to
     U_b = kap*(q0 - arow0) + W_corr @ e_k + (2/c2)*t + C_k
  where q0 = a K0 a, arow0 = a K0 1 come from M0 = A_aug K0 (col-tiled so
  kernel k / perm i stats live at partition 32k+i), e = the zeroed stripe
  K0[j, 384+j], and t is the per-permutation paired-sample sum. Pair and
  stripe squared distances are host-precomputed (d2p, 40KB) and share the
  device sqrt/exp path, so no pair-row gather DMA is needed. The U column
  falls out of q0/arow at the two augmented indicator rows (25 = all-X,
  26 = all-Y).

Final assembly transposes the per-partition stats onto rows with one PE
transpose matmul (identity rhs) and emits the whole [4, 26] result in one
DMA.
"""

import sys

import numpy as np

if "/opt/trn_rl_repo" not in sys.path:
    sys.path.insert(0, "/opt/trn_rl_repo")

import concourse.bacc as bacc
import concourse.bass as bass
import concourse.mybir as mybir
import concourse.tile as tile
from concourse import bass_utils

N = 384
NM = 768
D = 64
NPER = 200
NC = 8
PPC = NPER // NC  # 25
NP6 = 6 * NM      # 4608
WK = NP6 + 78     # 4686 = kernel-matrix cols + pair/stripe cols
C1 = float(N * (N - 1))
C2 = float(N * N)
KAP = np.float32(2.0 / C1 + 2.0 / C2)
CB1 = np.float32(1.0 / C1 + 2.0 / C2)
CB2 = np.float32(1.0 / C1)
TCO = np.float32(2.0 / C2)
IC1 = np.float32(1.0 / C1)
IC2 = np.float32(1.0 / C2)
KERNELS = ("gaussian", "laplacian", "gaussian", "laplacian")

F32 = mybir.dt.float32
F32R = mybir.dt.float32r
BF16 = mybir.dt.bfloat16
AF = mybir.ActivationFunctionType
ALU = mybir.AluOpType

# bigin f32 column layout
BG_ASTK = 0            # [128, 768] A_aug stacked per kernel group
BG_D2P = 768           # [128, 78] pair + stripe squared distances
BG_IDENT = 846         # [128, 128] identity (PE transpose rhs)
BG_AUX = 974           # [128, 10] per-kernel act scale/bias + eps
BG_AUX4 = 984          # [1, 8] row-0 diag constants
BG_W = 992
# atpb bf16 column layout
AB_ATP = 0             # [128, 192] A_aug^T chunks (32-padded)
AB_WCT = 192           # [128, 96] W_corr^T chunks (row 25 = ones -> se)
AB_W = 288


def _build():
    nc = bacc.Bacc("TRN2", target_bir_lowering=False, debug=False)
    with tile.TileContext(nc) as tc:
        with tc.tile_pool(name="dram", bufs=1, space="DRAM") as dram, \
             tc.tile_pool(name="io", bufs=1) as io, \
             tc.tile_pool(name="big", bufs=1) as big, \
             tc.tile_pool(name="kpool", bufs=4) as kpool, \
             tc.tile_pool(name="scr", bufs=1) as scr, \
             tc.tile_pool(name="sml", bufs=1) as sml:

            lrb_d = dram.tile([70, 2 * NM], BF16, kind="ExternalInput",
                              name="lrb", uniquify=False)
            bigin_d = dram.tile([128, BG_W], F32, kind="ExternalInput",
                                name="bigin", uniquify=False)
            atpb_d = dram.tile([128, AB_W], BF16, kind="ExternalInput",
                               name="atpb", uniquify=False)
            out_d = dram.tile([4, 1 + PPC], F32, kind="ExternalOutput",
                              name="out", uniquify=False)

            # ---- phase 0: input DMAs (L|R first: it gates the PE) ----
            lrb = io.tile([70, 2 * NM], BF16, name="lrb_sb")
            nc.sync.dma_start(out=lrb[:], in_=lrb_d[:])
            bigin = io.tile([128, BG_W], F32, name="bigin_sb")
            nc.sync.dma_start(out=bigin[:, BG_AUX:BG_W],
                              in_=bigin_d[:, BG_AUX:BG_W])
            atpb = io.tile([128, AB_W], BF16, name="atpb_sb")
            nc.sync.dma_start(out=atpb[:], in_=atpb_d[:])
            nc.sync.dma_start(out=bigin[:, 0:BG_AUX],
                              in_=bigin_d[:, 0:BG_AUX])

            astk = bigin[:, BG_ASTK:BG_ASTK + NM]
            d2pv = bigin[:, BG_D2P:BG_D2P + 78]
            identf = bigin[:, BG_IDENT:BG_IDENT + 128]
            aux = bigin[:, BG_AUX:BG_AUX + 10]
            aux4 = bigin[0:1, BG_AUX4:BG_AUX4 + 8]
            atp = atpb[:, AB_ATP:AB_ATP + 192]
            wct = atpb[:, AB_WCT:AB_WCT + 96]

            onesb = io.tile([128, 1], BF16, name="onesb_sb")
            nc.vector.memset(onesb[:], 1.0)
            onesf = io.tile([128, 1], F32, name="onesf_sb")
            nc.vector.memset(onesf[:], 1.0)
            fsrc = io.tile([128, 512], BF16, name="fsrc_sb")
            nc.vector.memset(fsrc[:], 0.0)

            # d2sb cols 0:4608 = clamped d2 row-tiles; 4608:4686 = host pair
            # d2, so ONE wide sqrt covers both.
            d2sb = big.tile([128, WK], F32, name="d2sb")
            dist = big.tile([128, WK], F32, name="dist_sb")
            kts = [kpool.tile([128, WK], BF16, name=f"kt{k}", tag="kt")
                   for k in range(4)]

            # Warm-up: loads the Exp act table while DMAs are in flight.
            warm = sml.tile([128, 1], F32, name="warm")
            nc.scalar.activation(warm[:], onesf[:], AF.Exp, scale=1.0,
                                 bias=onesf[:, 0:1])

            with tc.tile_pool(name="psM", bufs=1, space="PSUM") as psM:
                ps_m = psM.tile([128, NM], F32, name="ps_m")

                def m0(k, c, start, stop):
                    pr = slice(32 * k, 32 * k + 32)
                    lhsA = atp[:, 32 * c:32 * (c + 1)]
                    for s in (slice(0, 512), slice(512, NM)):
                        nc.tensor.matmul(ps_m[pr, s], lhsA,
                                         kts[k][:, NM * c + s.start:
                                                NM * c + s.stop],
                                         start=start, stop=stop,
                                         tile_position=(0, 32 * k),
                                         skip_group_check=True)

                # Warm the PE p-state while the input DMAs are in flight;
                # ps_m is re-zeroed by every M0 group's start flag.
                for _ in range(9):
                    nc.tensor.matmul(ps_m[:, 0:512], fsrc[:, 0:128],
                                     fsrc[:], start=True, stop=True,
                                     skip_group_check=True)

                with tc.tile_pool(name="psA", bufs=3, space="PSUM") as psA:
                    # ---- phase 1: d2 row-tiles on the PE (f32r), gaussian
                    # exps straight from PSUM, DVE clamp into d2sb ----
                    for r in range(6):
                        ps_d2 = psA.tile([128, NM], F32, name=f"ps_d2_{r}",
                                         tag="d2")
                        # K=70 bf16: rows 0:64 Zt / -2Zt, rows 64:70 carry
                        # the ones/sq rank-2 terms, sq split hi+mid+lo and
                        # derived from the bf16 z so the diagonal cancels.
                        lhsZ = lrb[:, 128 * r:128 * (r + 1)]
                        for s in (slice(0, 512), slice(512, NM)):
                            nc.tensor.matmul(ps_d2[:, s], lhsZ,
                                             lrb[:, NM + s.start:NM + s.stop],
                                             start=True, stop=True)
                        sl = slice(NM * r, NM * (r + 1))
                        for k in (0, 2):
                            nc.scalar.activation(
                                kts[k][:, sl], ps_d2[:], AF.Exp,
                                scale=aux[:, 2 * k:2 * k + 1],
                                bias=aux[:, 2 * k + 1:2 * k + 2])
                        nc.vector.tensor_scalar(
                            out=d2sb[:, sl], in0=ps_d2[:],
                            scalar1=0.0, scalar2=0.0,
                            op0=ALU.max, op1=ALU.add)
                    # pair d2 into the tail columns, then gaussian pair exps
                    nc.vector.tensor_copy(d2sb[:, NP6:WK], d2pv)
                    for k in (0, 2):
                        nc.scalar.activation(
                            kts[k][:, NP6:WK], d2sb[:, NP6:WK], AF.Exp,
                            scale=aux[:, 2 * k:2 * k + 1],
                            bias=aux[:, 2 * k + 1:2 * k + 2])

                    # ---- phase 2: gaussian M0 accumulation (col-tiled;
                    # kernel k's stats land at partitions 32k+i) ----
                    for c in range(6):
                        for k in (0, 2):
                            m0(k, c, start=(c == 0), stop=(c == 5))

                # ---- phase 3: dist = sqrt(d2 + 1e-12) incl pair cols,
                # laplacian exps (half-tiles, k-interleaved), lap M0 ----
                nc.scalar.activation(dist[:], d2sb[:], AF.Sqrt,
                                     scale=1.0, bias=aux[:, 8:9])

                with tc.tile_pool(name="psS", bufs=1, space="PSUM") as psS:
                    ps_tr = psS.tile([1, 384], F32, name="ps_tr")
                    ps_corr = psS.tile([128, 1], F32, name="ps_corr")
                    ps_T = psS.tile([1, 512], F32, name="ps_T")
                    ps_fill = psS.tile([128, 512], F32, name="ps_fill")

                    stk = sml.tile([128, 4], F32, name="stk")
                    sB = scr.tile([128, NM], F32, name="sB", tag="sB")
                    sP = scr.tile([128, NM], F32, name="sP", tag="sP")
                    trow = sml.tile([1, 4 * PPC], F32, name="trow")

                    def fill(n):
                        # keep the PE p-state hot across exp-wait gaps
                        for _ in range(n):
                            nc.tensor.matmul(ps_fill[:, 0:512],
                                             fsrc[:, 0:128], fsrc[:],
                                             start=True, stop=True,
                                             skip_group_check=True)

                    def corr_k(k):
                        for c in range(3):
                            nc.tensor.matmul(
                                ps_corr[32 * k:32 * k + 32, 0:1],
                                wct[:, 32 * c:32 * (c + 1)],
                                kts[k][:, NP6 + 75 + c:NP6 + 76 + c],
                                start=(c == 0), stop=(c == 2),
                                tile_position=(0, 32 * k),
                                skip_group_check=True)

                    def trow_mm(k):
                        nc.tensor.matmul(ps_tr[0:1, 96 * k:96 * k + 75],
                                         onesb[:, 0:1],
                                         kts[k][:, NP6:NP6 + 75],
                                         start=True, stop=True)

                    def trow_red(k):
                        nc.vector.tensor_reduce(
                            trow[0:1, PPC * k:PPC * (k + 1)],
                            ps_tr[0:1, 96 * k:96 * k + 75].rearrange(
                                "o (p t) -> o p t", t=3),
                            axis=mybir.AxisListType.X, op=ALU.add)

                    # gaussian t-term + corrections (early; PE idles anyway)
                    for k in (0, 2):
                        trow_mm(k)
                        corr_k(k)
                        trow_red(k)

                    # laplacian exps in half-tiles so M0 can chase them
                    H = 3 * NM  # 2304
                    for k in (1, 3):
                        nc.scalar.activation(
                            kts[k][:, 0:H], dist[:, 0:H], AF.Exp,
                            scale=aux[:, 2 * k:2 * k + 1],
                            bias=aux[:, 9:10])
                    for k in (1, 3):
                        nc.scalar.activation(
                            kts[k][:, H:WK], dist[:, H:WK], AF.Exp,
                            scale=aux[:, 2 * k:2 * k + 1],
                            bias=aux[:, 9:10])
                    fill(24)
                    for k in (1, 3):
                        for c in range(3):
                            m0(k, c, start=(c == 0), stop=False)
                    fill(6)
                    for c in range(3, 6):
                        m0(1, c, start=False, stop=(c == 5))
                    trow_mm(1)
                    corr_k(1)
                    trow_red(1)
                    fill(2)
                    for c in range(3, 6):
                        m0(3, c, start=False, stop=(c == 5))
                    trow_mm(3)
                    corr_k(3)
                    trow_red(3)
                    nc.vector.tensor_scalar_mul(trow[:], trow[:], float(TCO))
                    nc.vector.tensor_copy(stk[:, 3:4], ps_corr[:])
                    nc.tensor.matmul(ps_T[0:1, 384:512], stk[:, 3:4], identf,
                                     is_transpose=True, start=True, stop=True)

                    # ---- combined row stats: q0 on DVE, arow on Scalar ----
                    nc.vector.tensor_tensor(out=sB[:], in0=ps_m[:],
                                            in1=astk[:], op=ALU.mult)
                    nc.vector.tensor_reduce(stk[:, 1:2], sB[:],
                                            axis=mybir.AxisListType.X,
                                            op=ALU.add)
                    nc.scalar.activation(sP[:], ps_m[:], AF.Copy,
                                         scale=float(KAP), bias=0.0,
                                         accum_out=stk[:, 2:3])
                    for j in (1, 2):
                        nc.tensor.matmul(ps_T[0:1, 128 * j:128 * (j + 1)],
                                         stk[:, j:j + 1], identf,
                                         is_transpose=True,
                                         start=True, stop=True)
                    u0 = sml.tile([128, 1], F32, name="u0")
                    nc.vector.tensor_tensor(out=u0[:], in0=stk[:, 1:2],
                                            in1=stk[:, 2:3], op=ALU.subtract)
                    nc.vector.tensor_tensor(out=stk[:, 0:1], in0=u0[:],
                                            in1=ps_corr[:], op=ALU.add)
                    nc.tensor.matmul(ps_T[0:1, 0:128], stk[:, 0:1], identf,
                                     is_transpose=True, start=True, stop=True)
                    frow = sml.tile([1, 384], F32, name="frow")
                    nc.vector.tensor_copy(frow[:], ps_T[0:1, 128:512])

                    # ---- U stats on transposed rows (all kap-scaled) ----
                    def fr(base, step=32, count=4):
                        ap = frow[0:1, base:base + 1]
                        return bass.AP(ap.tensor, ap.offset,
                                       [ap.ap[0], [step, count]])

                    IC1K = float(IC1 / KAP)
                    XXv = fr(25)         # kap * 1_X K0 1_X
                    YYv = fr(26)         # kap * 1_Y K0 1_Y
                    a25v = fr(128 + 25)  # kap * 1_X K0 1
                    a26v = fr(128 + 26)  # kap * 1_Y K0 1
                    sevv = fr(256 + 25)  # kap * se_k (kap row of W_corr)
                    s12 = sml.tile([1, 4], F32, name="s12")
                    nc.vector.tensor_tensor(out=s12[:], in0=a25v, in1=a26v,
                                            op=ALU.add)
                    nc.vector.tensor_tensor(out=s12[:], in0=s12[:], in1=sevv,
                                            op=ALU.subtract)
                    nc.vector.tensor_tensor(out=s12[:], in0=s12[:],
                                            in1=aux4[0:1, 0:4],
                                            op=ALU.subtract)
                    ck = sml.tile([1, 4], F32, name="ck")
                    nc.vector.tensor_scalar_mul(ck[:], s12[:], IC1K)
                    u1 = sml.tile([1, 4], F32, name="u1")
                    nc.vector.tensor_tensor(out=u1[:], in0=XXv, in1=YYv,
                                            op=ALU.add)
                    nc.vector.tensor_tensor(out=u1[:], in0=u1[:],
                                            in1=aux4[0:1, 0:4],
                                            op=ALU.subtract)
                    nc.vector.tensor_scalar_mul(u1[:], u1[:], IC1K)
                    u2 = sml.tile([1, 4], F32, name="u2")
                    nc.vector.tensor_tensor(out=u2[:], in0=a25v, in1=XXv,
                                            op=ALU.subtract)
                    nc.vector.tensor_tensor(out=u2[:], in0=u2[:], in1=sevv,
                                            op=ALU.subtract)
                    nc.vector.tensor_scalar_mul(u2[:], u2[:],
                                                float(-2.0 * IC2 / KAP))

                    # ---- contiguous [1, 104] result, one DMA ----
                    uball = sml.tile([1, 4 * (1 + PPC)], F32, name="uball")
                    uball0 = uball[0:1, 0:1]
                    uFv = bass.AP(uball0.tensor, uball0.offset,
                                  [uball0.ap[0], [1 + PPC, 4]])
                    nc.vector.tensor_tensor(out=uFv, in0=u1[:], in1=u2[:],
                                            op=ALU.add)
                    ubv = bass.AP(uball0.tensor, uball0.offset + 1,
                                  [uball0.ap[0], [1 + PPC, 4], [1, PPC]])
                    ub_src = ps_T[0:1, 0:128].rearrange(
                        "o (k p) -> o k p", p=32)
                    ckap = ck[0:1, 0:1]
                    ck_b = bass.AP(ckap.tensor, ckap.offset,
                                   [ckap.ap[0], [1, 4], [0, PPC]])
                    nc.vector.tensor_tensor(out=ubv,
                                            in0=ub_src[0:1, :, 0:PPC],
                                            in1=ck_b, op=ALU.add)
                    trow_v = trow[0:1, :].rearrange("o (k p) -> o k p", p=PPC)
                    nc.vector.tensor_tensor(out=ubv, in0=ubv, in1=trow_v,
                                            op=ALU.add)
                    nc.gpsimd.dma_start(
                        out=out_d[:, :],
                        in_=uball[0:1, :].rearrange("o (k w) -> o k w",
                                                    w=1 + PPC))

    nc.compile()
    return nc


def _host_prep(X, Y, bandwidths, perms):
    X = np.ascontiguousarray(X, np.float32)
    Y = np.ascontiguousarray(Y, np.float32)
    perms = np.ascontiguousarray(perms, np.int32)
    import ml_dtypes

    Zf = np.concatenate([X, Y], 0)                  # [768, 64]
    Ztb = Zf.T.astype(ml_dtypes.bfloat16)           # [64, 768] device dtype
    Zb64 = Ztb.astype(np.float64).T                 # bf16-rounded z, f64
    sqb = np.einsum("ij,ij->i", Zb64, Zb64)         # exact device row norms
    Z64 = Zf.astype(np.float64)
    sq64 = np.einsum("ij,ij->i", Z64, Z64)
    sqhi = sqb.astype(np.float32).astype(ml_dtypes.bfloat16)
    r1 = sqb - sqhi.astype(np.float64)
    sqmid = r1.astype(np.float32).astype(ml_dtypes.bfloat16)
    r2 = r1 - sqmid.astype(np.float64)
    sqlo = r2.astype(np.float32).astype(ml_dtypes.bfloat16)
    lrb = np.zeros((70, 2 * NM), ml_dtypes.bfloat16)
    lrb[0:D, 0:NM] = Ztb
    lrb[0:D, NM:] = (-2.0 * Ztb.astype(np.float32)).astype(ml_dtypes.bfloat16)
    lrb[D:D + 3, 0:NM] = 1.0
    lrb[D + 3, 0:NM] = sqhi
    lrb[D + 4, 0:NM] = sqmid
    lrb[D + 5, 0:NM] = sqlo
    lrb[D, NM:] = sqhi
    lrb[D + 1, NM:] = sqmid
    lrb[D + 2, NM:] = sqlo
    lrb[D + 3:D + 6, NM:] = 1.0

    b = np.asarray(bandwidths, np.float64)
    gs = (-1.0 / (b * b)).astype(np.float32)
    ls = (-1.0 / b).astype(np.float32)
    aux = np.zeros((128, 10), np.float32)
    BETA = 0.01
    aux[:, 8] = BETA
    d0c = np.zeros(4, np.float64)
    for k, kern in enumerate(KERNELS):
        if kern == "gaussian":
            aux[:, 2 * k] = gs[k]
            aux[:, 2 * k + 1] = (gs[k].astype(np.float64) * 1e-12
                                 ).astype(np.float32)
            d0c[k] = np.exp(-1e-12 / (b[k] * b[k]))
        else:
            aux[:, 2 * k] = ls[k]
            d0c[k] = np.exp(-np.sqrt(BETA) / b[k])
    aux4 = np.zeros(8, np.float32)
    aux4[0:4] = (768.0 * d0c * float(KAP)).astype(np.float32)

    maps = []
    for cid in range(NC):
        pm = perms[cid * PPC:(cid + 1) * PPC]
        A = np.zeros((27, NM), np.float32)
        A[np.arange(PPC)[:, None], pm[:, :N]] = 1
        A[25, :N] = 1
        A[26, N:] = 1
        astk = np.zeros((128, NM), np.float32)
        for k in range(4):
            astk[32 * k:32 * k + 27] = A * KAP
        atp = np.zeros((128, 6 * 32), np.float32)
        for c in range(6):
            atp[:, 32 * c:32 * c + 27] = A[:, 128 * c:128 * (c + 1)].T
        A1 = A[:PPC, :N]
        A2 = A[:PPC, N:]
        Wc = (-KAP * (A1 * A2) + CB1 * A1 + CB2 * A2).astype(np.float32)
        wct = np.zeros((128, 3 * 32), np.float32)
        for c in range(3):
            wct[:, 32 * c:32 * c + PPC] = Wc[:, 128 * c:128 * (c + 1)].T
            wct[:, 32 * c + 25] = KAP  # kap*se_k extraction row

        # pair + stripe squared distances, f64 on host
        pX = pm[:, :N].astype(np.int64).ravel()
        pY = pm[:, N:].astype(np.int64).ravel()
        j = np.arange(N)
        pa = np.concatenate([pX, j])
        pb = np.concatenate([pY, N + j])
        d2pair = (sq64[pa] + sq64[pb]
                  - 2.0 * np.einsum("ij,ij->i", Z64[pa], Z64[pb]))
        d2pair = np.maximum(d2pair, 0.0) + 1e-12
        stripe = pY == pX + N
        d2pair[:N * PPC][stripe] = 1e12  # zeroed-stripe pairs: f_k -> 0
        d2p = d2pair.astype(np.float32).reshape(78, 128).T  # [128, 78]

        bigin = np.zeros((128, BG_W), np.float32)
        bigin[:, BG_ASTK:BG_ASTK + NM] = astk
        bigin[:, BG_D2P:BG_D2P + 78] = d2p
        bigin[:, BG_IDENT:BG_IDENT + 128] = np.eye(128, dtype=np.float32)
        bigin[:, BG_AUX:BG_AUX + 10] = aux
        bigin[0, BG_AUX4:BG_AUX4 + 8] = aux4
        atpb = np.zeros((128, AB_W), np.float32)
        atpb[:, AB_ATP:AB_ATP + 192] = atp
        atpb[:, AB_WCT:AB_WCT + 96] = wct
        maps.append(dict(lrb=lrb, bigin=bigin,
                         atpb=atpb.astype(ml_dtypes.bfloat16)))
    return maps


_NC_CACHE = None


def _get_nc():
    global _NC_CACHE
    if _NC_CACHE is None:
        _NC_CACHE = _build()
    return _NC_CACHE


def kernel(X, Y, bandwidths, perms):
    nc = _get_nc()
    in_maps = _host_prep(X, Y, bandwidths, perms)
    res = bass_utils.run_bass_kernel_spmd(nc, in_maps, list(range(NC)))
    full = np.zeros((4, 1 + NPER), np.float32)
    full[:, 0] = res.results[0]["out"][:, 0]
    for cid in range(NC):
        full[:, 1 + cid * PPC:1 + (cid + 1) * PPC] = \
            res.results[cid]["out"][:, 1:]
    return full


# revision 23
# speedup vs baseline: 1.0708x; 1.0230x over previous
"""Trainium2 Bass kernel for the 4-kernel MMD permutation test (nn_DUAL_78237124264373).

Math (per core, 25 of the 200 permutations; everything else replicated):
  Z = [X; Y] (768 x 64). The full squared-distance matrix lands in PSUM as a
  single rank-66 matmul d2 = L^T R with L = [Zt; 1; sq], R = [-2 Zt; sq; 1]
  (sq folded in on the host), so the gaussian kernels exp straight out of
  PSUM with per-kernel scalar scale/bias only. The laplacian kernels go
  through a DVE clamp -> one wide Scalar sqrt -> exp.
  With a_p the X-half indicator of permutation p, every U_b entry reduces to
     U_b = kap*(q0 - arow0) + W_corr @ e_k + (2/c2)*t + C_k
  where q0 = a K0 a, arow0 = a K0 1 come from M0 = A_aug K0 (col-tiled so
  kernel k / perm i stats live at partition 32k+i), e = the zeroed stripe
  K0[j, 384+j], and t is the per-permutation paired-sample sum. Pair and
  stripe squared distances are host-precomputed (d2p, 40KB) and share the
  device sqrt/exp path, so no pair-row gather DMA is needed. The U column
  falls out of q0/arow at the two augmented indicator rows (25 = all-X,
  26 = all-Y).

Final assembly transposes the per-partition stats onto rows with one PE
transpose matmul (identity rhs) and emits the whole [4, 26] result in one
DMA.
"""

import sys

import numpy as np

if "/opt/trn_rl_repo" not in sys.path:
    sys.path.insert(0, "/opt/trn_rl_repo")

import concourse.bacc as bacc
import concourse.bass as bass
import concourse.mybir as mybir
import concourse.tile as tile
from concourse import bass_utils

N = 384
NM = 768
D = 64
NPER = 200
NC = 8
PPC = NPER // NC  # 25
NP6 = 6 * NM      # 4608
WK = NP6 + 78     # 4686 = kernel-matrix cols + pair/stripe cols
C1 = float(N * (N - 1))
C2 = float(N * N)
KAP = np.float32(2.0 / C1 + 2.0 / C2)
CB1 = np.float32(1.0 / C1 + 2.0 / C2)
CB2 = np.float32(1.0 / C1)
TCO = np.float32(2.0 / C2)
IC1 = np.float32(1.0 / C1)
IC2 = np.float32(1.0 / C2)
KERNELS = ("gaussian", "laplacian", "gaussian", "laplacian")

F32 = mybir.dt.float32
F32R = mybir.dt.float32r
BF16 = mybir.dt.bfloat16
AF = mybir.ActivationFunctionType
ALU = mybir.AluOpType

# bigin f32 column layout
BG_ASTK = 0            # [128, 768] A_aug stacked per kernel group
BG_D2P = 768           # [128, 78] pair + stripe squared distances
BG_IDENT = 846         # [128, 128] identity (PE transpose rhs)
BG_AUX = 974           # [128, 10] per-kernel act scale/bias + eps
BG_AUX4 = 984          # [1, 8] row-0 diag constants
BG_W = 992
# atpb bf16 column layout
AB_ATP = 0             # [128, 192] A_aug^T chunks (32-padded)
AB_WCT = 192           # [128, 96] W_corr^T chunks (row 25 = ones -> se)
AB_W = 288


def _build():
    nc = bacc.Bacc("TRN2", target_bir_lowering=False, debug=False)
    with tile.TileContext(nc) as tc:
        with tc.tile_pool(name="dram", bufs=1, space="DRAM") as dram, \
             tc.tile_pool(name="io", bufs=1) as io, \
             tc.tile_pool(name="big", bufs=1) as big, \
             tc.tile_pool(name="kpool", bufs=4) as kpool, \
             tc.tile_pool(name="scr", bufs=1) as scr, \
             tc.tile_pool(name="sml", bufs=1) as sml:

            lrb_d = dram.tile([70, 2 * NM], BF16, kind="ExternalInput",
                              name="lrb", uniquify=False)
            bigin_d = dram.tile([128, BG_W], F32, kind="ExternalInput",
                                name="bigin", uniquify=False)
            atpb_d = dram.tile([128, AB_W], BF16, kind="ExternalInput",
                               name="atpb", uniquify=False)
            out_d = dram.tile([4, 1 + PPC], F32, kind="ExternalOutput",
                              name="out", uniquify=False)

            # ---- phase 0: input DMAs (L|R first: it gates the PE) ----
            lrb = io.tile([70, 2 * NM], BF16, name="lrb_sb")
            nc.sync.dma_start(out=lrb[:], in_=lrb_d[:])
            bigin = io.tile([128, BG_W], F32, name="bigin_sb")
            nc.sync.dma_start(out=bigin[:, BG_AUX:BG_W],
                              in_=bigin_d[:, BG_AUX:BG_W])
            atpb = io.tile([128, AB_W], BF16, name="atpb_sb")
            nc.sync.dma_start(out=atpb[:], in_=atpb_d[:])
            nc.sync.dma_start(out=bigin[:, 0:BG_AUX],
                              in_=bigin_d[:, 0:BG_AUX])

            astk = bigin[:, BG_ASTK:BG_ASTK + NM]
            d2pv = bigin[:, BG_D2P:BG_D2P + 78]
            identf = bigin[:, BG_IDENT:BG_IDENT + 128]
            aux = bigin[:, BG_AUX:BG_AUX + 10]
            aux4 = bigin[0:1, BG_AUX4:BG_AUX4 + 8]
            atp = atpb[:, AB_ATP:AB_ATP + 192]
            wct = atpb[:, AB_WCT:AB_WCT + 96]

            onesb = io.tile([128, 1], BF16, name="onesb_sb")
            nc.vector.memset(onesb[:], 1.0)
            onesf = io.tile([128, 1], F32, name="onesf_sb")
            nc.vector.memset(onesf[:], 1.0)
            fsrc = io.tile([128, 512], BF16, name="fsrc_sb")
            nc.vector.memset(fsrc[:], 0.0)

            # d2sb cols 0:4608 = clamped d2 row-tiles; 4608:4686 = host pair
            # d2, so ONE wide sqrt covers both.
            d2sb = big.tile([128, WK], F32, name="d2sb")
            dist = big.tile([128, WK], F32, name="dist_sb")
            kts = [kpool.tile([128, WK], BF16, name=f"kt{k}", tag="kt")
                   for k in range(4)]

            # Warm-up: loads the Exp act table while DMAs are in flight.
            warm = sml.tile([128, 1], F32, name="warm")
            nc.scalar.activation(warm[:], onesf[:], AF.Exp, scale=1.0,
                                 bias=onesf[:, 0:1])

            with tc.tile_pool(name="psM", bufs=1, space="PSUM") as psM:
                ps_m = psM.tile([128, NM], F32, name="ps_m")

                def m0(k, c, start, stop):
                    pr = slice(32 * k, 32 * k + 32)
                    lhsA = atp[:, 32 * c:32 * (c + 1)]
                    for s in (slice(0, 512), slice(512, NM)):
                        nc.tensor.matmul(ps_m[pr, s], lhsA,
                                         kts[k][:, NM * c + s.start:
                                                NM * c + s.stop],
                                         start=start, stop=stop,
                                         tile_position=(0, 32 * k),
                                         skip_group_check=True)

                # Warm the PE p-state while the input DMAs are in flight;
                # ps_m is re-zeroed by every M0 group's start flag.
                for _ in range(4):
                    nc.tensor.matmul(ps_m[:, 0:512], fsrc[:, 0:128],
                                     fsrc[:], start=True, stop=True,
                                     skip_group_check=True)

                with tc.tile_pool(name="psA", bufs=3, space="PSUM") as psA:
                    # ---- phase 1: d2 row-tiles on the PE (f32r), gaussian
                    # exps straight from PSUM, DVE clamp into d2sb ----
                    for r in range(6):
                        ps_d2 = psA.tile([128, NM], F32, name=f"ps_d2_{r}",
                                         tag="d2")
                        # K=70 bf16: rows 0:64 Zt / -2Zt, rows 64:70 carry
                        # the ones/sq rank-2 terms, sq split hi+mid+lo and
                        # derived from the bf16 z so the diagonal cancels.
                        lhsZ = lrb[:, 128 * r:128 * (r + 1)]
                        for s in (slice(0, 512), slice(512, NM)):
                            nc.tensor.matmul(ps_d2[:, s], lhsZ,
                                             lrb[:, NM + s.start:NM + s.stop],
                                             start=True, stop=True)
                        sl = slice(NM * r, NM * (r + 1))
                        for k in (0, 2):
                            nc.scalar.activation(
                                kts[k][:, sl], ps_d2[:], AF.Exp,
                                scale=aux[:, 2 * k:2 * k + 1],
                                bias=aux[:, 2 * k + 1:2 * k + 2])
                        nc.vector.tensor_scalar(
                            out=d2sb[:, sl], in0=ps_d2[:],
                            scalar1=0.0, scalar2=0.0,
                            op0=ALU.max, op1=ALU.add)
                    # pair d2 into the tail columns, then gaussian pair exps
                    nc.vector.tensor_copy(d2sb[:, NP6:WK], d2pv)
                    for k in (0, 2):
                        nc.scalar.activation(
                            kts[k][:, NP6:WK], d2sb[:, NP6:WK], AF.Exp,
                            scale=aux[:, 2 * k:2 * k + 1],
                            bias=aux[:, 2 * k + 1:2 * k + 2])

                    # ---- phase 2: gaussian M0 accumulation (col-tiled;
                    # kernel k's stats land at partitions 32k+i) ----
                    for c in range(6):
                        for k in (0, 2):
                            m0(k, c, start=(c == 0), stop=(c == 5))

                # ---- phase 3: dist = sqrt(d2 + 1e-12) incl pair cols,
                # laplacian exps (half-tiles, k-interleaved), lap M0 ----
                nc.scalar.activation(dist[:], d2sb[:], AF.Sqrt,
                                     scale=1.0, bias=aux[:, 8:9])

                with tc.tile_pool(name="psS", bufs=1, space="PSUM") as psS:
                    ps_tr = psS.tile([1, 384], F32, name="ps_tr")
                    ps_corr = psS.tile([128, 1], F32, name="ps_corr")
                    ps_T = psS.tile([1, 512], F32, name="ps_T")
                    ps_fill = psS.tile([128, 512], F32, name="ps_fill")

                    stk = sml.tile([128, 4], F32, name="stk")
                    sB = scr.tile([128, NM], F32, name="sB", tag="sB")
                    sP = scr.tile([128, NM], F32, name="sP", tag="sP")
                    trow = sml.tile([1, 4 * PPC], F32, name="trow")

                    def fill(n):
                        # keep the PE p-state hot across exp-wait gaps
                        for _ in range(n):
                            nc.tensor.matmul(ps_fill[:, 0:512],
                                             fsrc[:, 0:128], fsrc[:],
                                             start=True, stop=True,
                                             skip_group_check=True)

                    def corr_k(k):
                        for c in range(3):
                            nc.tensor.matmul(
                                ps_corr[32 * k:32 * k + 32, 0:1],
                                wct[:, 32 * c:32 * (c + 1)],
                                kts[k][:, NP6 + 75 + c:NP6 + 76 + c],
                                start=(c == 0), stop=(c == 2),
                                tile_position=(0, 32 * k),
                                skip_group_check=True)

                    def trow_mm(k):
                        nc.tensor.matmul(ps_tr[0:1, 96 * k:96 * k + 75],
                                         onesb[:, 0:1],
                                         kts[k][:, NP6:NP6 + 75],
                                         start=True, stop=True)

                    def trow_red(k):
                        nc.vector.tensor_reduce(
                            trow[0:1, PPC * k:PPC * (k + 1)],
                            ps_tr[0:1, 96 * k:96 * k + 75].rearrange(
                                "o (p t) -> o p t", t=3),
                            axis=mybir.AxisListType.X, op=ALU.add)

                    # gaussian t-term + corrections (early; PE idles anyway)
                    for k in (0, 2):
                        trow_mm(k)
                        corr_k(k)
                        trow_red(k)

                    # laplacian exps in half-tiles so M0 can chase them
                    H = 3 * NM  # 2304
                    for k in (1, 3):
                        nc.scalar.activation(
                            kts[k][:, 0:H], dist[:, 0:H], AF.Exp,
                            scale=aux[:, 2 * k:2 * k + 1],
                            bias=aux[:, 9:10])
                    nc.scalar.activation(
                        kts[1][:, H:WK], dist[:, H:WK], AF.Exp,
                        scale=aux[:, 2:3], bias=aux[:, 9:10])
                    for sl in (slice(H, H + NM), slice(H + NM, H + 2 * NM),
                               slice(H + 2 * NM, WK)):
                        nc.scalar.activation(
                            kts[3][:, sl], dist[:, sl], AF.Exp,
                            scale=aux[:, 6:7], bias=aux[:, 9:10])
                    fill(24)
                    for k in (1, 3):
                        for c in range(3):
                            m0(k, c, start=(c == 0), stop=False)
                    fill(6)
                    for c in range(3, 6):
                        m0(1, c, start=False, stop=(c == 5))
                    trow_mm(1)
                    corr_k(1)
                    trow_red(1)
                    fill(2)
                    m0(3, 3, start=False, stop=False)
                    m0(3, 4, start=False, stop=False)
                    m0(3, 5, start=False, stop=True)
                    trow_mm(3)
                    corr_k(3)
                    trow_red(3)
                    nc.vector.tensor_scalar_mul(trow[:], trow[:], float(TCO))
                    nc.vector.tensor_copy(stk[:, 3:4], ps_corr[:])
                    nc.tensor.matmul(ps_T[0:1, 384:512], stk[:, 3:4], identf,
                                     is_transpose=True, start=True, stop=True)

                    # ---- combined row stats: q0 on DVE, arow on Scalar ----
                    nc.vector.tensor_tensor(out=sB[:], in0=ps_m[:],
                                            in1=astk[:], op=ALU.mult)
                    nc.vector.tensor_reduce(stk[:, 1:2], sB[:],
                                            axis=mybir.AxisListType.X,
                                            op=ALU.add)
                    nc.scalar.activation(sP[:], ps_m[:], AF.Copy,
                                         scale=float(KAP), bias=0.0,
                                         accum_out=stk[:, 2:3])
                    for j in (1, 2):
                        nc.tensor.matmul(ps_T[0:1, 128 * j:128 * (j + 1)],
                                         stk[:, j:j + 1], identf,
                                         is_transpose=True,
                                         start=True, stop=True)
                    u0 = sml.tile([128, 1], F32, name="u0")
                    nc.vector.tensor_tensor(out=u0[:], in0=stk[:, 1:2],
                                            in1=stk[:, 2:3], op=ALU.subtract)
                    nc.vector.tensor_tensor(out=stk[:, 0:1], in0=u0[:],
                                            in1=ps_corr[:], op=ALU.add)
                    nc.tensor.matmul(ps_T[0:1, 0:128], stk[:, 0:1], identf,
                                     is_transpose=True, start=True, stop=True)
                    frow = sml.tile([1, 384], F32, name="frow")
                    nc.vector.tensor_copy(frow[:], ps_T[0:1, 128:512])

                    # ---- U stats on transposed rows (all kap-scaled) ----
                    def fr(base, step=32, count=4):
                        ap = frow[0:1, base:base + 1]
                        return bass.AP(ap.tensor, ap.offset,
                                       [ap.ap[0], [step, count]])

                    IC1K = float(IC1 / KAP)
                    XXv = fr(25)         # kap * 1_X K0 1_X
                    YYv = fr(26)         # kap * 1_Y K0 1_Y
                    a25v = fr(128 + 25)  # kap * 1_X K0 1
                    a26v = fr(128 + 26)  # kap * 1_Y K0 1
                    sevv = fr(256 + 25)  # kap * se_k (kap row of W_corr)
                    s12 = sml.tile([1, 4], F32, name="s12")
                    nc.vector.tensor_tensor(out=s12[:], in0=a25v, in1=a26v,
                                            op=ALU.add)
                    nc.vector.tensor_tensor(out=s12[:], in0=s12[:], in1=sevv,
                                            op=ALU.subtract)
                    nc.vector.tensor_tensor(out=s12[:], in0=s12[:],
                                            in1=aux4[0:1, 0:4],
                                            op=ALU.subtract)
                    ck = sml.tile([1, 4], F32, name="ck")
                    nc.vector.tensor_scalar_mul(ck[:], s12[:], IC1K)
                    u1 = sml.tile([1, 4], F32, name="u1")
                    nc.vector.tensor_tensor(out=u1[:], in0=XXv, in1=YYv,
                                            op=ALU.add)
                    nc.vector.tensor_tensor(out=u1[:], in0=u1[:],
                                            in1=aux4[0:1, 0:4],
                                            op=ALU.subtract)
                    nc.vector.tensor_scalar_mul(u1[:], u1[:], IC1K)
                    u2 = sml.tile([1, 4], F32, name="u2")
                    nc.vector.tensor_tensor(out=u2[:], in0=a25v, in1=XXv,
                                            op=ALU.subtract)
                    nc.vector.tensor_tensor(out=u2[:], in0=u2[:], in1=sevv,
                                            op=ALU.subtract)
                    nc.vector.tensor_scalar_mul(u2[:], u2[:],
                                                float(-2.0 * IC2 / KAP))

                    # ---- contiguous [1, 104] result, one DMA ----
                    uball = sml.tile([1, 4 * (1 + PPC)], F32, name="uball")
                    uball0 = uball[0:1, 0:1]
                    uFv = bass.AP(uball0.tensor, uball0.offset,
                                  [uball0.ap[0], [1 + PPC, 4]])
                    nc.vector.tensor_tensor(out=uFv, in0=u1[:], in1=u2[:],
                                            op=ALU.add)
                    ubv = bass.AP(uball0.tensor, uball0.offset + 1,
                                  [uball0.ap[0], [1 + PPC, 4], [1, PPC]])
                    ub_src = ps_T[0:1, 0:128].rearrange(
                        "o (k p) -> o k p", p=32)
                    ckap = ck[0:1, 0:1]
                    ck_b = bass.AP(ckap.tensor, ckap.offset,
                                   [ckap.ap[0], [1, 4], [0, PPC]])
                    nc.vector.tensor_tensor(out=ubv,
                                            in0=ub_src[0:1, :, 0:PPC],
                                            in1=ck_b, op=ALU.add)
                    trow_v = trow[0:1, :].rearrange("o (k p) -> o k p", p=PPC)
                    nc.vector.tensor_tensor(out=ubv, in0=ubv, in1=trow_v,
                                            op=ALU.add)
                    nc.gpsimd.dma_start(
                        out=out_d[:, :],
                        in_=uball[0:1, :].rearrange("o (k w) -> o k w",
                                                    w=1 + PPC))

    nc.compile()
    return nc


def _host_prep(X, Y, bandwidths, perms):
    X = np.ascontiguousarray(X, np.float32)
    Y = np.ascontiguousarray(Y, np.float32)
    perms = np.ascontiguousarray(perms, np.int32)
    import ml_dtypes

    Zf = np.concatenate([X, Y], 0)                  # [768, 64]
    Ztb = Zf.T.astype(ml_dtypes.bfloat16)           # [64, 768] device dtype
    Zb64 = Ztb.astype(np.float64).T                 # bf16-rounded z, f64
    sqb = np.einsum("ij,ij->i", Zb64, Zb64)         # exact device row norms
    Z64 = Zf.astype(np.float64)
    sq64 = np.einsum("ij,ij->i", Z64, Z64)
    sqhi = sqb.astype(np.float32).astype(ml_dtypes.bfloat16)
    r1 = sqb - sqhi.astype(np.float64)
    sqmid = r1.astype(np.float32).astype(ml_dtypes.bfloat16)
    r2 = r1 - sqmid.astype(np.float64)
    sqlo = r2.astype(np.float32).astype(ml_dtypes.bfloat16)
    lrb = np.zeros((70, 2 * NM), ml_dtypes.bfloat16)
    lrb[0:D, 0:NM] = Ztb
    lrb[0:D, NM:] = (-2.0 * Ztb.astype(np.float32)).astype(ml_dtypes.bfloat16)
    lrb[D:D + 3, 0:NM] = 1.0
    lrb[D + 3, 0:NM] = sqhi
    lrb[D + 4, 0:NM] = sqmid
    lrb[D + 5, 0:NM] = sqlo
    lrb[D, NM:] = sqhi
    lrb[D + 1, NM:] = sqmid
    lrb[D + 2, NM:] = sqlo
    lrb[D + 3:D + 6, NM:] = 1.0

    b = np.asarray(bandwidths, np.float64)
    gs = (-1.0 / (b * b)).astype(np.float32)
    ls = (-1.0 / b).astype(np.float32)
    aux = np.zeros((128, 10), np.float32)
    BETA = 0.01
    aux[:, 8] = BETA
    d0c = np.zeros(4, np.float64)
    for k, kern in enumerate(KERNELS):
        if kern == "gaussian":
            aux[:, 2 * k] = gs[k]
            aux[:, 2 * k + 1] = (gs[k].astype(np.float64) * 1e-12
                                 ).astype(np.float32)
            d0c[k] = np.exp(-1e-12 / (b[k] * b[k]))
        else:
            aux[:, 2 * k] = ls[k]
            d0c[k] = np.exp(-np.sqrt(BETA) / b[k])
    aux4 = np.zeros(8, np.float32)
    aux4[0:4] = (768.0 * d0c * float(KAP)).astype(np.float32)

    maps = []
    for cid in range(NC):
        pm = perms[cid * PPC:(cid + 1) * PPC]
        A = np.zeros((27, NM), np.float32)
        A[np.arange(PPC)[:, None], pm[:, :N]] = 1
        A[25, :N] = 1
        A[26, N:] = 1
        astk = np.zeros((128, NM), np.float32)
        for k in range(4):
            astk[32 * k:32 * k + 27] = A * KAP
        atp = np.zeros((128, 6 * 32), np.float32)
        for c in range(6):
            atp[:, 32 * c:32 * c + 27] = A[:, 128 * c:128 * (c + 1)].T
        A1 = A[:PPC, :N]
        A2 = A[:PPC, N:]
        Wc = (-KAP * (A1 * A2) + CB1 * A1 + CB2 * A2).astype(np.float32)
        wct = np.zeros((128, 3 * 32), np.float32)
        for c in range(3):
            wct[:, 32 * c:32 * c + PPC] = Wc[:, 128 * c:128 * (c + 1)].T
            wct[:, 32 * c + 25] = KAP  # kap*se_k extraction row

        # pair + stripe squared distances, f64 on host
        pX = pm[:, :N].astype(np.int64).ravel()
        pY = pm[:, N:].astype(np.int64).ravel()
        j = np.arange(N)
        pa = np.concatenate([pX, j])
        pb = np.concatenate([pY, N + j])
        d2pair = (sq64[pa] + sq64[pb]
                  - 2.0 * np.einsum("ij,ij->i", Z64[pa], Z64[pb]))
        d2pair = np.maximum(d2pair, 0.0) + 1e-12
        stripe = pY == pX + N
        d2pair[:N * PPC][stripe] = 1e12  # zeroed-stripe pairs: f_k -> 0
        d2p = d2pair.astype(np.float32).reshape(78, 128).T  # [128, 78]

        bigin = np.zeros((128, BG_W), np.float32)
        bigin[:, BG_ASTK:BG_ASTK + NM] = astk
        bigin[:, BG_D2P:BG_D2P + 78] = d2p
        bigin[:, BG_IDENT:BG_IDENT + 128] = np.eye(128, dtype=np.float32)
        bigin[:, BG_AUX:BG_AUX + 10] = aux
        bigin[0, BG_AUX4:BG_AUX4 + 8] = aux4
        atpb = np.zeros((128, AB_W), np.float32)
        atpb[:, AB_ATP:AB_ATP + 192] = atp
        atpb[:, AB_WCT:AB_WCT + 96] = wct
        maps.append(dict(lrb=lrb, bigin=bigin,
                         atpb=atpb.astype(ml_dtypes.bfloat16)))
    return maps


_NC_CACHE = None


def _get_nc():
    global _NC_CACHE
    if _NC_CACHE is None:
        _NC_CACHE = _build()
    return _NC_CACHE


def kernel(X, Y, bandwidths, perms):
    nc = _get_nc()
    in_maps = _host_prep(X, Y, bandwidths, perms)
    res = bass_utils.run_bass_kernel_spmd(nc, in_maps, list(range(NC)))
    full = np.zeros((4, 1 + NPER), np.float32)
    full[:, 0] = res.results[0]["out"][:, 0]
    for cid in range(NC):
        full[:, 1 + cid * PPC:1 + (cid + 1) * PPC] = \
            res.results[cid]["out"][:, 1:]
    return full


# revision 25
# speedup vs baseline: 1.0715x; 1.0007x over previous
"""Trainium2 Bass kernel for the 4-kernel MMD permutation test (nn_DUAL_78237124264373).

Math (per core, 25 of the 200 permutations; everything else replicated):
  Z = [X; Y] (768 x 64). The full squared-distance matrix lands in PSUM as a
  single rank-66 matmul d2 = L^T R with L = [Zt; 1; sq], R = [-2 Zt; sq; 1]
  (sq folded in on the host), so the gaussian kernels exp straight out of
  PSUM with per-kernel scalar scale/bias only. The laplacian kernels go
  through a DVE clamp -> one wide Scalar sqrt -> exp.
  With a_p the X-half indicator of permutation p, every U_b entry reduces to
     U_b = kap*(q0 - arow0) + W_corr @ e_k + (2/c2)*t + C_k
  where q0 = a K0 a, arow0 = a K0 1 come from M0 = A_aug K0 (col-tiled so
  kernel k / perm i stats live at partition 32k+i), e = the zeroed stripe
  K0[j, 384+j], and t is the per-permutation paired-sample sum. Pair and
  stripe squared distances are host-precomputed (d2p, 40KB) and share the
  device sqrt/exp path, so no pair-row gather DMA is needed. The U column
  falls out of q0/arow at the two augmented indicator rows (25 = all-X,
  26 = all-Y).

Final assembly transposes the per-partition stats onto rows with one PE
transpose matmul (identity rhs) and emits the whole [4, 26] result in one
DMA.
"""

import sys

import numpy as np

if "/opt/trn_rl_repo" not in sys.path:
    sys.path.insert(0, "/opt/trn_rl_repo")

import concourse.bacc as bacc
import concourse.bass as bass
import concourse.mybir as mybir
import concourse.tile as tile
from concourse import bass_utils

N = 384
NM = 768
D = 64
NPER = 200
NC = 8
PPC = NPER // NC  # 25
NP6 = 6 * NM      # 4608
WK = NP6 + 78     # 4686 = kernel-matrix cols + pair/stripe cols
C1 = float(N * (N - 1))
C2 = float(N * N)
KAP = np.float32(2.0 / C1 + 2.0 / C2)
CB1 = np.float32(1.0 / C1 + 2.0 / C2)
CB2 = np.float32(1.0 / C1)
TCO = np.float32(2.0 / C2)
IC1 = np.float32(1.0 / C1)
IC2 = np.float32(1.0 / C2)
KERNELS = ("gaussian", "laplacian", "gaussian", "laplacian")

F32 = mybir.dt.float32
F32R = mybir.dt.float32r
BF16 = mybir.dt.bfloat16
AF = mybir.ActivationFunctionType
ALU = mybir.AluOpType

# bigin f32 column layout
BG_ASTK = 0            # [128, 768] A_aug stacked per kernel group
BG_D2P = 768           # [128, 78] pair + stripe squared distances
BG_IDENT = 846         # [128, 128] identity (PE transpose rhs)
BG_AUX = 974           # [128, 10] per-kernel act scale/bias + eps
BG_AUX4 = 984          # [1, 8] row-0 diag constants
BG_W = 992
# atpb bf16 column layout
AB_ATP = 0             # [128, 192] A_aug^T chunks (32-padded)
AB_WCT = 192           # [128, 96] W_corr^T chunks (row 25 = ones -> se)
AB_W = 288


def _build():
    nc = bacc.Bacc("TRN2", target_bir_lowering=False, debug=False)
    with tile.TileContext(nc) as tc:
        with tc.tile_pool(name="dram", bufs=1, space="DRAM") as dram, \
             tc.tile_pool(name="io", bufs=1) as io, \
             tc.tile_pool(name="big", bufs=1) as big, \
             tc.tile_pool(name="kpool", bufs=4) as kpool, \
             tc.tile_pool(name="scr", bufs=1) as scr, \
             tc.tile_pool(name="sml", bufs=1) as sml:

            lrb_d = dram.tile([70, 2 * NM], BF16, kind="ExternalInput",
                              name="lrb", uniquify=False)
            bigin_d = dram.tile([128, BG_W], F32, kind="ExternalInput",
                                name="bigin", uniquify=False)
            atpb_d = dram.tile([128, AB_W], BF16, kind="ExternalInput",
                               name="atpb", uniquify=False)
            out_d = dram.tile([4, 1 + PPC], F32, kind="ExternalOutput",
                              name="out", uniquify=False)

            # ---- phase 0: input DMAs (L|R first: it gates the PE) ----
            lrb = io.tile([70, 2 * NM], BF16, name="lrb_sb")
            nc.scalar.dma_start(out=lrb[:], in_=lrb_d[:])
            bigin = io.tile([128, BG_W], F32, name="bigin_sb")
            nc.gpsimd.dma_start(out=bigin[:, BG_AUX:BG_W],
                                in_=bigin_d[:, BG_AUX:BG_W])
            atpb = io.tile([128, AB_W], BF16, name="atpb_sb")
            nc.sync.dma_start(out=atpb[:], in_=atpb_d[:])
            nc.sync.dma_start(out=bigin[:, 0:BG_AUX],
                              in_=bigin_d[:, 0:BG_AUX])

            astk = bigin[:, BG_ASTK:BG_ASTK + NM]
            d2pv = bigin[:, BG_D2P:BG_D2P + 78]
            identf = bigin[:, BG_IDENT:BG_IDENT + 128]
            aux = bigin[:, BG_AUX:BG_AUX + 10]
            aux4 = bigin[0:1, BG_AUX4:BG_AUX4 + 8]
            atp = atpb[:, AB_ATP:AB_ATP + 192]
            wct = atpb[:, AB_WCT:AB_WCT + 96]

            onesb = io.tile([128, 1], BF16, name="onesb_sb")
            nc.vector.memset(onesb[:], 1.0)
            onesf = io.tile([128, 1], F32, name="onesf_sb")
            nc.vector.memset(onesf[:], 1.0)
            fsrc = io.tile([128, 512], BF16, name="fsrc_sb")
            nc.vector.memset(fsrc[:], 0.0)

            # d2sb cols 0:4608 = clamped d2 row-tiles; 4608:4686 = host pair
            # d2, so ONE wide sqrt covers both.
            d2sb = big.tile([128, WK], F32, name="d2sb")
            dist = big.tile([128, WK], F32, name="dist_sb")
            kts = [kpool.tile([128, WK], BF16, name=f"kt{k}", tag="kt")
                   for k in range(4)]

            # Warm-up: loads the Exp act table while DMAs are in flight.
            warm = sml.tile([128, 1], F32, name="warm")
            nc.scalar.activation(warm[:], onesf[:], AF.Exp, scale=1.0,
                                 bias=onesf[:, 0:1])

            with tc.tile_pool(name="psM", bufs=1, space="PSUM") as psM:
                ps_m = psM.tile([128, NM], F32, name="ps_m")

                def m0(k, c, start, stop):
                    pr = slice(32 * k, 32 * k + 32)
                    lhsA = atp[:, 32 * c:32 * (c + 1)]
                    for s in (slice(0, 512), slice(512, NM)):
                        nc.tensor.matmul(ps_m[pr, s], lhsA,
                                         kts[k][:, NM * c + s.start:
                                                NM * c + s.stop],
                                         start=start, stop=stop,
                                         tile_position=(0, 32 * k),
                                         skip_group_check=True)

                # Warm the PE p-state while the input DMAs are in flight;
                # ps_m is re-zeroed by every M0 group's start flag.
                for _ in range(4):
                    nc.tensor.matmul(ps_m[:, 0:512], fsrc[:, 0:128],
                                     fsrc[:], start=True, stop=True,
                                     skip_group_check=True)

                with tc.tile_pool(name="psA", bufs=3, space="PSUM") as psA:
                    # ---- phase 1: d2 row-tiles on the PE (f32r), gaussian
                    # exps straight from PSUM, DVE clamp into d2sb ----
                    for r in range(6):
                        ps_d2 = psA.tile([128, NM], F32, name=f"ps_d2_{r}",
                                         tag="d2")
                        # K=70 bf16: rows 0:64 Zt / -2Zt, rows 64:70 carry
                        # the ones/sq rank-2 terms, sq split hi+mid+lo and
                        # derived from the bf16 z so the diagonal cancels.
                        lhsZ = lrb[:, 128 * r:128 * (r + 1)]
                        for s in (slice(0, 512), slice(512, NM)):
                            nc.tensor.matmul(ps_d2[:, s], lhsZ,
                                             lrb[:, NM + s.start:NM + s.stop],
                                             start=True, stop=True)
                        sl = slice(NM * r, NM * (r + 1))
                        for k in (0, 2):
                            nc.scalar.activation(
                                kts[k][:, sl], ps_d2[:], AF.Exp,
                                scale=aux[:, 2 * k:2 * k + 1],
                                bias=aux[:, 2 * k + 1:2 * k + 2])
                        nc.vector.tensor_scalar(
                            out=d2sb[:, sl], in0=ps_d2[:],
                            scalar1=0.0, scalar2=0.0,
                            op0=ALU.max, op1=ALU.add)
                        if r == 0:
                            # pair d2 into the tail cols (host data, early)
                            nc.vector.tensor_copy(d2sb[:, NP6:WK], d2pv)
                        if r in (2, 4):
                            # pair exps absorb the tile-production gaps
                            k = 0 if r == 2 else 2
                            nc.scalar.activation(
                                kts[k][:, NP6:WK], d2sb[:, NP6:WK], AF.Exp,
                                scale=aux[:, 2 * k:2 * k + 1],
                                bias=aux[:, 2 * k + 1:2 * k + 2])

                    # ---- phase 2: gaussian M0 accumulation (col-tiled;
                    # kernel k's stats land at partitions 32k+i) ----
                    for c in range(6):
                        for k in (0, 2):
                            m0(k, c, start=(c == 0), stop=(c == 5))

                # ---- phase 3: dist = sqrt(d2 + 1e-12) incl pair cols,
                # laplacian exps (half-tiles, k-interleaved), lap M0 ----
                nc.scalar.activation(dist[:], d2sb[:], AF.Sqrt,
                                     scale=1.0, bias=aux[:, 8:9])

                with tc.tile_pool(name="psS", bufs=1, space="PSUM") as psS:
                    ps_tr = psS.tile([1, 384], F32, name="ps_tr")
                    ps_corr = psS.tile([128, 1], F32, name="ps_corr")
                    ps_T = psS.tile([1, 512], F32, name="ps_T")
                    ps_fill = psS.tile([128, 512], F32, name="ps_fill")

                    stk = sml.tile([128, 4], F32, name="stk")
                    sB = scr.tile([128, NM], F32, name="sB", tag="sB")
                    sP = scr.tile([128, NM], F32, name="sP", tag="sP")
                    trow = sml.tile([1, 4 * PPC], F32, name="trow")

                    def fill(n):
                        # keep the PE p-state hot across exp-wait gaps
                        for _ in range(n):
                            nc.tensor.matmul(ps_fill[:, 0:512],
                                             fsrc[:, 0:128], fsrc[:],
                                             start=True, stop=True,
                                             skip_group_check=True)

                    def corr_k(k):
                        for c in range(3):
                            nc.tensor.matmul(
                                ps_corr[32 * k:32 * k + 32, 0:1],
                                wct[:, 32 * c:32 * (c + 1)],
                                kts[k][:, NP6 + 75 + c:NP6 + 76 + c],
                                start=(c == 0), stop=(c == 2),
                                tile_position=(0, 32 * k),
                                skip_group_check=True)

                    def trow_mm(k):
                        nc.tensor.matmul(ps_tr[0:1, 96 * k:96 * k + 75],
                                         onesb[:, 0:1],
                                         kts[k][:, NP6:NP6 + 75],
                                         start=True, stop=True)

                    def trow_red(k):
                        nc.vector.tensor_reduce(
                            trow[0:1, PPC * k:PPC * (k + 1)],
                            ps_tr[0:1, 96 * k:96 * k + 75].rearrange(
                                "o (p t) -> o p t", t=3),
                            axis=mybir.AxisListType.X, op=ALU.add)

                    # gaussian t-term + corrections (early; PE idles anyway)
                    for k in (0, 2):
                        trow_mm(k)
                        corr_k(k)
                        trow_red(k)

                    # laplacian exps in half-tiles so M0 can chase them
                    H = 3 * NM  # 2304
                    for k in (1, 3):
                        nc.scalar.activation(
                            kts[k][:, 0:H], dist[:, 0:H], AF.Exp,
                            scale=aux[:, 2 * k:2 * k + 1],
                            bias=aux[:, 9:10])
                    nc.scalar.activation(
                        kts[1][:, H:WK], dist[:, H:WK], AF.Exp,
                        scale=aux[:, 2:3], bias=aux[:, 9:10])
                    for sl in (slice(H, H + NM), slice(H + NM, H + 2 * NM),
                               slice(H + 2 * NM, WK)):
                        nc.scalar.activation(
                            kts[3][:, sl], dist[:, sl], AF.Exp,
                            scale=aux[:, 6:7], bias=aux[:, 9:10])
                    fill(24)
                    for k in (1, 3):
                        for c in range(3):
                            m0(k, c, start=(c == 0), stop=False)
                    fill(6)
                    for c in range(3, 6):
                        m0(1, c, start=False, stop=(c == 5))
                    trow_mm(1)
                    corr_k(1)
                    trow_red(1)
                    fill(2)
                    m0(3, 3, start=False, stop=False)
                    m0(3, 4, start=False, stop=False)
                    m0(3, 5, start=False, stop=True)
                    trow_mm(3)
                    corr_k(3)
                    trow_red(3)
                    nc.vector.tensor_scalar_mul(trow[:], trow[:], float(TCO))
                    nc.vector.tensor_copy(stk[:, 3:4], ps_corr[:])
                    nc.tensor.matmul(ps_T[0:1, 384:512], stk[:, 3:4], identf,
                                     is_transpose=True, start=True, stop=True)

                    # ---- combined row stats: q0 on DVE, arow on Scalar ----
                    nc.vector.tensor_tensor(out=sB[:], in0=ps_m[:],
                                            in1=astk[:], op=ALU.mult)
                    nc.vector.tensor_reduce(stk[:, 1:2], sB[:],
                                            axis=mybir.AxisListType.X,
                                            op=ALU.add)
                    nc.scalar.activation(sP[:], ps_m[:], AF.Copy,
                                         scale=float(KAP), bias=0.0,
                                         accum_out=stk[:, 2:3])
                    for j in (1, 2):
                        nc.tensor.matmul(ps_T[0:1, 128 * j:128 * (j + 1)],
                                         stk[:, j:j + 1], identf,
                                         is_transpose=True,
                                         start=True, stop=True)
                    u0 = sml.tile([128, 1], F32, name="u0")
                    nc.vector.tensor_tensor(out=u0[:], in0=stk[:, 1:2],
                                            in1=stk[:, 2:3], op=ALU.subtract)
                    nc.vector.tensor_tensor(out=stk[:, 0:1], in0=u0[:],
                                            in1=ps_corr[:], op=ALU.add)
                    nc.tensor.matmul(ps_T[0:1, 0:128], stk[:, 0:1], identf,
                                     is_transpose=True, start=True, stop=True)
                    frow = sml.tile([1, 384], F32, name="frow")
                    nc.vector.tensor_copy(frow[:], ps_T[0:1, 128:512])

                    # ---- U stats on transposed rows (all kap-scaled) ----
                    def fr(base, step=32, count=4):
                        ap = frow[0:1, base:base + 1]
                        return bass.AP(ap.tensor, ap.offset,
                                       [ap.ap[0], [step, count]])

                    IC1K = float(IC1 / KAP)
                    XXv = fr(25)         # kap * 1_X K0 1_X
                    YYv = fr(26)         # kap * 1_Y K0 1_Y
                    a25v = fr(128 + 25)  # kap * 1_X K0 1
                    a26v = fr(128 + 26)  # kap * 1_Y K0 1
                    sevv = fr(256 + 25)  # kap * se_k (kap row of W_corr)
                    s12 = sml.tile([1, 4], F32, name="s12")
                    nc.vector.tensor_tensor(out=s12[:], in0=a25v, in1=a26v,
                                            op=ALU.add)
                    nc.vector.tensor_tensor(out=s12[:], in0=s12[:], in1=sevv,
                                            op=ALU.subtract)
                    nc.vector.tensor_tensor(out=s12[:], in0=s12[:],
                                            in1=aux4[0:1, 0:4],
                                            op=ALU.subtract)
                    ck = sml.tile([1, 4], F32, name="ck")
                    nc.vector.tensor_scalar_mul(ck[:], s12[:], IC1K)
                    u1 = sml.tile([1, 4], F32, name="u1")
                    nc.vector.tensor_tensor(out=u1[:], in0=XXv, in1=YYv,
                                            op=ALU.add)
                    nc.vector.tensor_tensor(out=u1[:], in0=u1[:],
                                            in1=aux4[0:1, 0:4],
                                            op=ALU.subtract)
                    nc.vector.tensor_scalar_mul(u1[:], u1[:], IC1K)
                    u2 = sml.tile([1, 4], F32, name="u2")
                    nc.vector.tensor_tensor(out=u2[:], in0=a25v, in1=XXv,
                                            op=ALU.subtract)
                    nc.vector.tensor_tensor(out=u2[:], in0=u2[:], in1=sevv,
                                            op=ALU.subtract)
                    nc.vector.tensor_scalar_mul(u2[:], u2[:],
                                                float(-2.0 * IC2 / KAP))

                    # ---- contiguous [1, 104] result, one DMA ----
                    uball = sml.tile([1, 4 * (1 + PPC)], F32, name="uball")
                    uball0 = uball[0:1, 0:1]
                    uFv = bass.AP(uball0.tensor, uball0.offset,
                                  [uball0.ap[0], [1 + PPC, 4]])
                    nc.vector.tensor_tensor(out=uFv, in0=u1[:], in1=u2[:],
                                            op=ALU.add)
                    ubv = bass.AP(uball0.tensor, uball0.offset + 1,
                                  [uball0.ap[0], [1 + PPC, 4], [1, PPC]])
                    ub_src = ps_T[0:1, 0:128].rearrange(
                        "o (k p) -> o k p", p=32)
                    ckap = ck[0:1, 0:1]
                    ck_b = bass.AP(ckap.tensor, ckap.offset,
                                   [ckap.ap[0], [1, 4], [0, PPC]])
                    nc.vector.tensor_tensor(out=ubv,
                                            in0=ub_src[0:1, :, 0:PPC],
                                            in1=ck_b, op=ALU.add)
                    trow_v = trow[0:1, :].rearrange("o (k p) -> o k p", p=PPC)
                    nc.vector.tensor_tensor(out=ubv, in0=ubv, in1=trow_v,
                                            op=ALU.add)
                    nc.gpsimd.dma_start(
                        out=out_d[:, :],
                        in_=uball[0:1, :].rearrange("o (k w) -> o k w",
                                                    w=1 + PPC))

    nc.compile()
    return nc


def _host_prep(X, Y, bandwidths, perms):
    X = np.ascontiguousarray(X, np.float32)
    Y = np.ascontiguousarray(Y, np.float32)
    perms = np.ascontiguousarray(perms, np.int32)
    import ml_dtypes

    Zf = np.concatenate([X, Y], 0)                  # [768, 64]
    Ztb = Zf.T.astype(ml_dtypes.bfloat16)           # [64, 768] device dtype
    Zb64 = Ztb.astype(np.float64).T                 # bf16-rounded z, f64
    sqb = np.einsum("ij,ij->i", Zb64, Zb64)         # exact device row norms
    Z64 = Zf.astype(np.float64)
    sq64 = np.einsum("ij,ij->i", Z64, Z64)
    sqhi = sqb.astype(np.float32).astype(ml_dtypes.bfloat16)
    r1 = sqb - sqhi.astype(np.float64)
    sqmid = r1.astype(np.float32).astype(ml_dtypes.bfloat16)
    r2 = r1 - sqmid.astype(np.float64)
    sqlo = r2.astype(np.float32).astype(ml_dtypes.bfloat16)
    lrb = np.zeros((70, 2 * NM), ml_dtypes.bfloat16)
    lrb[0:D, 0:NM] = Ztb
    lrb[0:D, NM:] = (-2.0 * Ztb.astype(np.float32)).astype(ml_dtypes.bfloat16)
    lrb[D:D + 3, 0:NM] = 1.0
    lrb[D + 3, 0:NM] = sqhi
    lrb[D + 4, 0:NM] = sqmid
    lrb[D + 5, 0:NM] = sqlo
    lrb[D, NM:] = sqhi
    lrb[D + 1, NM:] = sqmid
    lrb[D + 2, NM:] = sqlo
    lrb[D + 3:D + 6, NM:] = 1.0

    b = np.asarray(bandwidths, np.float64)
    gs = (-1.0 / (b * b)).astype(np.float32)
    ls = (-1.0 / b).astype(np.float32)
    aux = np.zeros((128, 10), np.float32)
    BETA = 0.01
    aux[:, 8] = BETA
    d0c = np.zeros(4, np.float64)
    for k, kern in enumerate(KERNELS):
        if kern == "gaussian":
            aux[:, 2 * k] = gs[k]
            aux[:, 2 * k + 1] = (gs[k].astype(np.float64) * 1e-12
                                 ).astype(np.float32)
            d0c[k] = np.exp(-1e-12 / (b[k] * b[k]))
        else:
            aux[:, 2 * k] = ls[k]
            d0c[k] = np.exp(-np.sqrt(BETA) / b[k])
    aux4 = np.zeros(8, np.float32)
    aux4[0:4] = (768.0 * d0c * float(KAP)).astype(np.float32)

    maps = []
    for cid in range(NC):
        pm = perms[cid * PPC:(cid + 1) * PPC]
        A = np.zeros((27, NM), np.float32)
        A[np.arange(PPC)[:, None], pm[:, :N]] = 1
        A[25, :N] = 1
        A[26, N:] = 1
        astk = np.zeros((128, NM), np.float32)
        for k in range(4):
            astk[32 * k:32 * k + 27] = A * KAP
        atp = np.zeros((128, 6 * 32), np.float32)
        for c in range(6):
            atp[:, 32 * c:32 * c + 27] = A[:, 128 * c:128 * (c + 1)].T
        A1 = A[:PPC, :N]
        A2 = A[:PPC, N:]
        Wc = (-KAP * (A1 * A2) + CB1 * A1 + CB2 * A2).astype(np.float32)
        wct = np.zeros((128, 3 * 32), np.float32)
        for c in range(3):
            wct[:, 32 * c:32 * c + PPC] = Wc[:, 128 * c:128 * (c + 1)].T
            wct[:, 32 * c + 25] = KAP  # kap*se_k extraction row

        # pair + stripe squared distances, f64 on host
        pX = pm[:, :N].astype(np.int64).ravel()
        pY = pm[:, N:].astype(np.int64).ravel()
        j = np.arange(N)
        pa = np.concatenate([pX, j])
        pb = np.concatenate([pY, N + j])
        d2pair = (sq64[pa] + sq64[pb]
                  - 2.0 * np.einsum("ij,ij->i", Z64[pa], Z64[pb]))
        d2pair = np.maximum(d2pair, 0.0) + 1e-12
        stripe = pY == pX + N
        d2pair[:N * PPC][stripe] = 1e12  # zeroed-stripe pairs: f_k -> 0
        d2p = d2pair.astype(np.float32).reshape(78, 128).T  # [128, 78]

        bigin = np.zeros((128, BG_W), np.float32)
        bigin[:, BG_ASTK:BG_ASTK + NM] = astk
        bigin[:, BG_D2P:BG_D2P + 78] = d2p
        bigin[:, BG_IDENT:BG_IDENT + 128] = np.eye(128, dtype=np.float32)
        bigin[:, BG_AUX:BG_AUX + 10] = aux
        bigin[0, BG_AUX4:BG_AUX4 + 8] = aux4
        atpb = np.zeros((128, AB_W), np.float32)
        atpb[:, AB_ATP:AB_ATP + 192] = atp
        atpb[:, AB_WCT:AB_WCT + 96] = wct
        maps.append(dict(lrb=lrb, bigin=bigin,
                         atpb=atpb.astype(ml_dtypes.bfloat16)))
    return maps


_NC_CACHE = None


def _get_nc():
    global _NC_CACHE
    if _NC_CACHE is None:
        _NC_CACHE = _build()
    return _NC_CACHE


def kernel(X, Y, bandwidths, perms):
    nc = _get_nc()
    in_maps = _host_prep(X, Y, bandwidths, perms)
    res = bass_utils.run_bass_kernel_spmd(nc, in_maps, list(range(NC)))
    full = np.zeros((4, 1 + NPER), np.float32)
    full[:, 0] = res.results[0]["out"][:, 0]
    for cid in range(NC):
        full[:, 1 + cid * PPC:1 + (cid + 1) * PPC] = \
            res.results[cid]["out"][:, 1:]
    return full


# revision 26
# speedup vs baseline: 1.0851x; 1.0126x over previous
"""Trainium2 Bass kernel for the 4-kernel MMD permutation test (nn_DUAL_78237124264373).

Math (per core, 25 of the 200 permutations; everything else replicated):
  Z = [X; Y] (768 x 64). The full squared-distance matrix lands in PSUM as a
  single rank-66 matmul d2 = L^T R with L = [Zt; 1; sq], R = [-2 Zt; sq; 1]
  (sq folded in on the host), so the gaussian kernels exp straight out of
  PSUM with per-kernel scalar scale/bias only. The laplacian kernels go
  through a DVE clamp -> one wide Scalar sqrt -> exp.
  With a_p the X-half indicator of permutation p, every U_b entry reduces to
     U_b = kap*(q0 - arow0) + W_corr @ e_k + (2/c2)*t + C_k
  where q0 = a K0 a, arow0 = a K0 1 come from M0 = A_aug K0 (col-tiled so
  kernel k / perm i stats live at partition 32k+i), e = the zeroed stripe
  K0[j, 384+j], and t is the per-permutation paired-sample sum. Pair and
  stripe squared distances are host-precomputed (d2p, 40KB) and share the
  device sqrt/exp path, so no pair-row gather DMA is needed. The U column
  falls out of q0/arow at the two augmented indicator rows (25 = all-X,
  26 = all-Y).

Final assembly transposes the per-partition stats onto rows with one PE
transpose matmul (identity rhs) and emits the whole [4, 26] result in one
DMA.
"""

import sys

import numpy as np

if "/opt/trn_rl_repo" not in sys.path:
    sys.path.insert(0, "/opt/trn_rl_repo")

import concourse.bacc as bacc
import concourse.bass as bass
import concourse.mybir as mybir
import concourse.tile as tile
from concourse import bass_utils

N = 384
NM = 768
D = 64
NPER = 200
NC = 8
PPC = NPER // NC  # 25
NP6 = 6 * NM      # 4608
WK = NP6 + 78     # 4686 = kernel-matrix cols + pair/stripe cols
C1 = float(N * (N - 1))
C2 = float(N * N)
KAP = np.float32(2.0 / C1 + 2.0 / C2)
CB1 = np.float32(1.0 / C1 + 2.0 / C2)
CB2 = np.float32(1.0 / C1)
TCO = np.float32(2.0 / C2)
IC1 = np.float32(1.0 / C1)
IC2 = np.float32(1.0 / C2)
KERNELS = ("gaussian", "laplacian", "gaussian", "laplacian")

F32 = mybir.dt.float32
F32R = mybir.dt.float32r
BF16 = mybir.dt.bfloat16
AF = mybir.ActivationFunctionType
ALU = mybir.AluOpType

# bigin f32 column layout
BG_ASTK = 0            # [128, 768] A_aug stacked per kernel group
BG_D2P = 768           # [128, 78] pair + stripe squared distances
BG_IDENT = 846         # [128, 128] identity (PE transpose rhs)
BG_AUX = 974           # [128, 10] per-kernel act scale/bias + eps
BG_AUX4 = 984          # [1, 8] row-0 diag constants
BG_W = 992
# atpb bf16 column layout
AB_ATP = 0             # [128, 192] A_aug^T chunks (32-padded)
AB_WCT = 192           # [128, 96] W_corr^T chunks (row 25 = ones -> se)
AB_W = 288


def _build():
    nc = bacc.Bacc("TRN2", target_bir_lowering=False, debug=False)
    with tile.TileContext(nc) as tc:
        with tc.tile_pool(name="dram", bufs=1, space="DRAM") as dram, \
             tc.tile_pool(name="io", bufs=1) as io, \
             tc.tile_pool(name="big", bufs=1) as big, \
             tc.tile_pool(name="kpool", bufs=4) as kpool, \
             tc.tile_pool(name="scr", bufs=1) as scr, \
             tc.tile_pool(name="sml", bufs=1) as sml:

            lrb_d = dram.tile([70, 2 * NM], BF16, kind="ExternalInput",
                              name="lrb", uniquify=False)
            bigin_d = dram.tile([128, BG_W], F32, kind="ExternalInput",
                                name="bigin", uniquify=False)
            atpb_d = dram.tile([128, AB_W], BF16, kind="ExternalInput",
                               name="atpb", uniquify=False)
            out_d = dram.tile([4, 1 + PPC], F32, kind="ExternalOutput",
                              name="out", uniquify=False)

            # ---- phase 0: input DMAs (L|R first: it gates the PE) ----
            lrb = io.tile([70, 2 * NM], BF16, name="lrb_sb")
            nc.sync.dma_start(out=lrb[:], in_=lrb_d[:])
            bigin = io.tile([128, BG_W], F32, name="bigin_sb")
            nc.gpsimd.dma_start(out=bigin[:, BG_AUX:BG_W],
                                in_=bigin_d[:, BG_AUX:BG_W])
            atpb = io.tile([128, AB_W], BF16, name="atpb_sb")
            nc.sync.dma_start(out=atpb[:], in_=atpb_d[:])
            nc.sync.dma_start(out=bigin[:, 0:BG_AUX],
                              in_=bigin_d[:, 0:BG_AUX])

            astk = bigin[:, BG_ASTK:BG_ASTK + NM]
            d2pv = bigin[:, BG_D2P:BG_D2P + 78]
            identf = bigin[:, BG_IDENT:BG_IDENT + 128]
            aux = bigin[:, BG_AUX:BG_AUX + 10]
            aux4 = bigin[0:1, BG_AUX4:BG_AUX4 + 8]
            atp = atpb[:, AB_ATP:AB_ATP + 192]
            wct = atpb[:, AB_WCT:AB_WCT + 96]

            onesb = io.tile([128, 1], BF16, name="onesb_sb")
            nc.vector.memset(onesb[:], 1.0)
            onesf = io.tile([128, 1], F32, name="onesf_sb")
            nc.vector.memset(onesf[:], 1.0)
            fsrc = io.tile([128, 512], BF16, name="fsrc_sb")
            nc.vector.memset(fsrc[:], 0.0)

            # d2sb cols 0:4608 = clamped d2 row-tiles; 4608:4686 = host pair
            # d2, so ONE wide sqrt covers both.
            d2sb = big.tile([128, WK], F32, name="d2sb")
            dist = big.tile([128, WK], F32, name="dist_sb")
            kts = [kpool.tile([128, WK], BF16, name=f"kt{k}", tag="kt")
                   for k in range(4)]

            # Warm-up: loads the Exp act table while DMAs are in flight.
            warm = sml.tile([128, 1], F32, name="warm")
            nc.scalar.activation(warm[:], onesf[:], AF.Exp, scale=1.0,
                                 bias=onesf[:, 0:1])

            with tc.tile_pool(name="psM", bufs=1, space="PSUM") as psM:
                ps_m = psM.tile([128, NM], F32, name="ps_m")

                def m0(k, c, start, stop):
                    pr = slice(32 * k, 32 * k + 32)
                    lhsA = atp[:, 32 * c:32 * (c + 1)]
                    for s in (slice(0, 512), slice(512, NM)):
                        nc.tensor.matmul(ps_m[pr, s], lhsA,
                                         kts[k][:, NM * c + s.start:
                                                NM * c + s.stop],
                                         start=start, stop=stop,
                                         tile_position=(0, 32 * k),
                                         skip_group_check=True)

                # Warm the PE p-state while the input DMAs are in flight;
                # ps_m is re-zeroed by every M0 group's start flag.
                for _ in range(4):
                    nc.tensor.matmul(ps_m[:, 0:512], fsrc[:, 0:128],
                                     fsrc[:], start=True, stop=True,
                                     skip_group_check=True)

                with tc.tile_pool(name="psA", bufs=3, space="PSUM") as psA:
                    # ---- phase 1: d2 row-tiles on the PE (f32r), gaussian
                    # exps straight from PSUM, DVE clamp into d2sb ----
                    for r in range(6):
                        ps_d2 = psA.tile([128, NM], F32, name=f"ps_d2_{r}",
                                         tag="d2")
                        # K=70 bf16: rows 0:64 Zt / -2Zt, rows 64:70 carry
                        # the ones/sq rank-2 terms, sq split hi+mid+lo and
                        # derived from the bf16 z so the diagonal cancels.
                        lhsZ = lrb[:, 128 * r:128 * (r + 1)]
                        for s in (slice(0, 512), slice(512, NM)):
                            nc.tensor.matmul(ps_d2[:, s], lhsZ,
                                             lrb[:, NM + s.start:NM + s.stop],
                                             start=True, stop=True)
                        sl = slice(NM * r, NM * (r + 1))
                        for k in (0, 2):
                            nc.scalar.activation(
                                kts[k][:, sl], ps_d2[:], AF.Exp,
                                scale=aux[:, 2 * k:2 * k + 1],
                                bias=aux[:, 2 * k + 1:2 * k + 2])
                        nc.vector.tensor_scalar(
                            out=d2sb[:, sl], in0=ps_d2[:],
                            scalar1=0.0, scalar2=0.0,
                            op0=ALU.max, op1=ALU.add)
                        if r == 0:
                            # pair d2 into the tail cols (host data, early)
                            nc.vector.tensor_copy(d2sb[:, NP6:WK], d2pv)
                        if r in (2, 4):
                            # pair exps absorb the tile-production gaps
                            k = 0 if r == 2 else 2
                            nc.scalar.activation(
                                kts[k][:, NP6:WK], d2sb[:, NP6:WK], AF.Exp,
                                scale=aux[:, 2 * k:2 * k + 1],
                                bias=aux[:, 2 * k + 1:2 * k + 2])

                    # ---- phase 2: gaussian M0 accumulation (col-tiled;
                    # kernel k's stats land at partitions 32k+i) ----
                    for c in range(6):
                        for k in (0, 2):
                            m0(k, c, start=(c == 0), stop=(c == 5))

                # ---- phase 3: dist = sqrt(d2 + 1e-12) incl pair cols,
                # laplacian exps (half-tiles, k-interleaved), lap M0 ----
                nc.scalar.activation(dist[:], d2sb[:], AF.Sqrt,
                                     scale=1.0, bias=aux[:, 8:9])

                with tc.tile_pool(name="psS", bufs=1, space="PSUM") as psS:
                    ps_tr = psS.tile([1, 384], F32, name="ps_tr")
                    ps_corr = psS.tile([128, 1], F32, name="ps_corr")
                    ps_T = psS.tile([1, 512], F32, name="ps_T")
                    ps_fill = psS.tile([128, 512], F32, name="ps_fill")

                    stk = sml.tile([128, 4], F32, name="stk")
                    sB = scr.tile([128, NM], F32, name="sB", tag="sB")
                    sP = scr.tile([128, NM], F32, name="sP", tag="sP")
                    trow = sml.tile([1, 4 * PPC], F32, name="trow")

                    def fill(n):
                        # keep the PE p-state hot across exp-wait gaps
                        for _ in range(n):
                            nc.tensor.matmul(ps_fill[:, 0:512],
                                             fsrc[:, 0:128], fsrc[:],
                                             start=True, stop=True,
                                             skip_group_check=True)

                    def corr_k(k):
                        for c in range(3):
                            nc.tensor.matmul(
                                ps_corr[32 * k:32 * k + 32, 0:1],
                                wct[:, 32 * c:32 * (c + 1)],
                                kts[k][:, NP6 + 75 + c:NP6 + 76 + c],
                                start=(c == 0), stop=(c == 2),
                                tile_position=(0, 32 * k),
                                skip_group_check=True)

                    def trow_mm(k):
                        nc.tensor.matmul(ps_tr[0:1, 96 * k:96 * k + 75],
                                         onesb[:, 0:1],
                                         kts[k][:, NP6:NP6 + 75],
                                         start=True, stop=True)

                    def trow_red(k):
                        nc.vector.tensor_reduce(
                            trow[0:1, PPC * k:PPC * (k + 1)],
                            ps_tr[0:1, 96 * k:96 * k + 75].rearrange(
                                "o (p t) -> o p t", t=3),
                            axis=mybir.AxisListType.X, op=ALU.add)

                    # gaussian t-term + corrections (early; PE idles anyway)
                    for k in (0, 2):
                        trow_mm(k)
                        corr_k(k)
                        trow_red(k)

                    # laplacian exps in half-tiles so M0 can chase them
                    H = 3 * NM  # 2304
                    for k in (1, 3):
                        nc.scalar.activation(
                            kts[k][:, 0:H], dist[:, 0:H], AF.Exp,
                            scale=aux[:, 2 * k:2 * k + 1],
                            bias=aux[:, 9:10])
                    nc.scalar.activation(
                        kts[1][:, H:WK], dist[:, H:WK], AF.Exp,
                        scale=aux[:, 2:3], bias=aux[:, 9:10])
                    for sl in (slice(H, H + NM), slice(H + NM, H + 2 * NM),
                               slice(H + 2 * NM, WK)):
                        nc.scalar.activation(
                            kts[3][:, sl], dist[:, sl], AF.Exp,
                            scale=aux[:, 6:7], bias=aux[:, 9:10])
                    fill(24)
                    for k in (1, 3):
                        for c in range(3):
                            m0(k, c, start=(c == 0), stop=False)
                    fill(6)
                    for c in range(3, 6):
                        m0(1, c, start=False, stop=(c == 5))
                    trow_mm(1)
                    corr_k(1)
                    trow_red(1)
                    fill(2)
                    m0(3, 3, start=False, stop=False)
                    m0(3, 4, start=False, stop=False)
                    m0(3, 5, start=False, stop=True)
                    trow_mm(3)
                    corr_k(3)
                    trow_red(3)
                    nc.vector.tensor_scalar_mul(trow[:], trow[:], float(TCO))
                    nc.vector.tensor_copy(stk[:, 3:4], ps_corr[:])
                    nc.tensor.matmul(ps_T[0:1, 384:512], stk[:, 3:4], identf,
                                     is_transpose=True, start=True, stop=True)

                    # ---- combined row stats: q0 on DVE, arow on Scalar ----
                    nc.vector.tensor_tensor(out=sB[:], in0=ps_m[:],
                                            in1=astk[:], op=ALU.mult)
                    nc.vector.tensor_reduce(stk[:, 1:2], sB[:],
                                            axis=mybir.AxisListType.X,
                                            op=ALU.add)
                    nc.scalar.activation(sP[:], ps_m[:], AF.Copy,
                                         scale=float(KAP), bias=0.0,
                                         accum_out=stk[:, 2:3])
                    for j in (1, 2):
                        nc.tensor.matmul(ps_T[0:1, 128 * j:128 * (j + 1)],
                                         stk[:, j:j + 1], identf,
                                         is_transpose=True,
                                         start=True, stop=True)
                    u0 = sml.tile([128, 1], F32, name="u0")
                    nc.vector.tensor_tensor(out=u0[:], in0=stk[:, 1:2],
                                            in1=stk[:, 2:3], op=ALU.subtract)
                    nc.vector.tensor_tensor(out=stk[:, 0:1], in0=u0[:],
                                            in1=ps_corr[:], op=ALU.add)
                    nc.tensor.matmul(ps_T[0:1, 0:128], stk[:, 0:1], identf,
                                     is_transpose=True, start=True, stop=True)
                    frow = sml.tile([1, 384], F32, name="frow")
                    nc.vector.tensor_copy(frow[:], ps_T[0:1, 128:512])

                    # ---- U stats on transposed rows (all kap-scaled) ----
                    def fr(base, step=32, count=4):
                        ap = frow[0:1, base:base + 1]
                        return bass.AP(ap.tensor, ap.offset,
                                       [ap.ap[0], [step, count]])

                    IC1K = float(IC1 / KAP)
                    XXv = fr(25)         # kap * 1_X K0 1_X
                    YYv = fr(26)         # kap * 1_Y K0 1_Y
                    a25v = fr(128 + 25)  # kap * 1_X K0 1
                    a26v = fr(128 + 26)  # kap * 1_Y K0 1
                    sevv = fr(256 + 25)  # kap * se_k (kap row of W_corr)
                    s12 = sml.tile([1, 4], F32, name="s12")
                    nc.vector.tensor_tensor(out=s12[:], in0=a25v, in1=a26v,
                                            op=ALU.add)
                    nc.vector.tensor_tensor(out=s12[:], in0=s12[:], in1=sevv,
                                            op=ALU.subtract)
                    nc.vector.tensor_tensor(out=s12[:], in0=s12[:],
                                            in1=aux4[0:1, 0:4],
                                            op=ALU.subtract)
                    ck = sml.tile([1, 4], F32, name="ck")
                    nc.vector.tensor_scalar_mul(ck[:], s12[:], IC1K)
                    u1 = sml.tile([1, 4], F32, name="u1")
                    nc.vector.tensor_tensor(out=u1[:], in0=XXv, in1=YYv,
                                            op=ALU.add)
                    nc.vector.tensor_tensor(out=u1[:], in0=u1[:],
                                            in1=aux4[0:1, 0:4],
                                            op=ALU.subtract)
                    nc.vector.tensor_scalar_mul(u1[:], u1[:], IC1K)
                    u2 = sml.tile([1, 4], F32, name="u2")
                    nc.vector.tensor_tensor(out=u2[:], in0=a25v, in1=XXv,
                                            op=ALU.subtract)
                    nc.vector.tensor_tensor(out=u2[:], in0=u2[:], in1=sevv,
                                            op=ALU.subtract)
                    nc.vector.tensor_scalar_mul(u2[:], u2[:],
                                                float(-2.0 * IC2 / KAP))

                    # ---- contiguous [1, 104] result, one DMA ----
                    uball = sml.tile([1, 4 * (1 + PPC)], F32, name="uball")
                    uball0 = uball[0:1, 0:1]
                    uFv = bass.AP(uball0.tensor, uball0.offset,
                                  [uball0.ap[0], [1 + PPC, 4]])
                    nc.vector.tensor_tensor(out=uFv, in0=u1[:], in1=u2[:],
                                            op=ALU.add)
                    ubv = bass.AP(uball0.tensor, uball0.offset + 1,
                                  [uball0.ap[0], [1 + PPC, 4], [1, PPC]])
                    ub_src = ps_T[0:1, 0:128].rearrange(
                        "o (k p) -> o k p", p=32)
                    ckap = ck[0:1, 0:1]
                    ck_b = bass.AP(ckap.tensor, ckap.offset,
                                   [ckap.ap[0], [1, 4], [0, PPC]])
                    nc.vector.tensor_tensor(out=ubv,
                                            in0=ub_src[0:1, :, 0:PPC],
                                            in1=ck_b, op=ALU.add)
                    trow_v = trow[0:1, :].rearrange("o (k p) -> o k p", p=PPC)
                    nc.vector.tensor_tensor(out=ubv, in0=ubv, in1=trow_v,
                                            op=ALU.add)
                    nc.gpsimd.dma_start(
                        out=out_d[:, :],
                        in_=uball[0:1, :].rearrange("o (k w) -> o k w",
                                                    w=1 + PPC))

    nc.compile()
    return nc


def _host_prep(X, Y, bandwidths, perms):
    X = np.ascontiguousarray(X, np.float32)
    Y = np.ascontiguousarray(Y, np.float32)
    perms = np.ascontiguousarray(perms, np.int32)
    import ml_dtypes

    Zf = np.concatenate([X, Y], 0)                  # [768, 64]
    Ztb = Zf.T.astype(ml_dtypes.bfloat16)           # [64, 768] device dtype
    Zb64 = Ztb.astype(np.float64).T                 # bf16-rounded z, f64
    sqb = np.einsum("ij,ij->i", Zb64, Zb64)         # exact device row norms
    Z64 = Zf.astype(np.float64)
    sq64 = np.einsum("ij,ij->i", Z64, Z64)
    sqhi = sqb.astype(np.float32).astype(ml_dtypes.bfloat16)
    r1 = sqb - sqhi.astype(np.float64)
    sqmid = r1.astype(np.float32).astype(ml_dtypes.bfloat16)
    r2 = r1 - sqmid.astype(np.float64)
    sqlo = r2.astype(np.float32).astype(ml_dtypes.bfloat16)
    lrb = np.zeros((70, 2 * NM), ml_dtypes.bfloat16)
    lrb[0:D, 0:NM] = Ztb
    lrb[0:D, NM:] = (-2.0 * Ztb.astype(np.float32)).astype(ml_dtypes.bfloat16)
    lrb[D:D + 3, 0:NM] = 1.0
    lrb[D + 3, 0:NM] = sqhi
    lrb[D + 4, 0:NM] = sqmid
    lrb[D + 5, 0:NM] = sqlo
    lrb[D, NM:] = sqhi
    lrb[D + 1, NM:] = sqmid
    lrb[D + 2, NM:] = sqlo
    lrb[D + 3:D + 6, NM:] = 1.0

    b = np.asarray(bandwidths, np.float64)
    gs = (-1.0 / (b * b)).astype(np.float32)
    ls = (-1.0 / b).astype(np.float32)
    aux = np.zeros((128, 10), np.float32)
    BETA = 0.01
    aux[:, 8] = BETA
    d0c = np.zeros(4, np.float64)
    for k, kern in enumerate(KERNELS):
        if kern == "gaussian":
            aux[:, 2 * k] = gs[k]
            aux[:, 2 * k + 1] = (gs[k].astype(np.float64) * 1e-12
                                 ).astype(np.float32)
            d0c[k] = np.exp(-1e-12 / (b[k] * b[k]))
        else:
            aux[:, 2 * k] = ls[k]
            d0c[k] = np.exp(-np.sqrt(BETA) / b[k])
    aux4 = np.zeros(8, np.float32)
    aux4[0:4] = (768.0 * d0c * float(KAP)).astype(np.float32)

    maps = []
    for cid in range(NC):
        pm = perms[cid * PPC:(cid + 1) * PPC]
        A = np.zeros((27, NM), np.float32)
        A[np.arange(PPC)[:, None], pm[:, :N]] = 1
        A[25, :N] = 1
        A[26, N:] = 1
        astk = np.zeros((128, NM), np.float32)
        for k in range(4):
            astk[32 * k:32 * k + 27] = A * KAP
        atp = np.zeros((128, 6 * 32), np.float32)
        for c in range(6):
            atp[:, 32 * c:32 * c + 27] = A[:, 128 * c:128 * (c + 1)].T
        A1 = A[:PPC, :N]
        A2 = A[:PPC, N:]
        Wc = (-KAP * (A1 * A2) + CB1 * A1 + CB2 * A2).astype(np.float32)
        wct = np.zeros((128, 3 * 32), np.float32)
        for c in range(3):
            wct[:, 32 * c:32 * c + PPC] = Wc[:, 128 * c:128 * (c + 1)].T
            wct[:, 32 * c + 25] = KAP  # kap*se_k extraction row

        # pair + stripe squared distances, f64 on host
        pX = pm[:, :N].astype(np.int64).ravel()
        pY = pm[:, N:].astype(np.int64).ravel()
        j = np.arange(N)
        pa = np.concatenate([pX, j])
        pb = np.concatenate([pY, N + j])
        d2pair = (sq64[pa] + sq64[pb]
                  - 2.0 * np.einsum("ij,ij->i", Z64[pa], Z64[pb]))
        d2pair = np.maximum(d2pair, 0.0) + 1e-12
        stripe = pY == pX + N
        d2pair[:N * PPC][stripe] = 1e12  # zeroed-stripe pairs: f_k -> 0
        d2p = d2pair.astype(np.float32).reshape(78, 128).T  # [128, 78]

        bigin = np.zeros((128, BG_W), np.float32)
        bigin[:, BG_ASTK:BG_ASTK + NM] = astk
        bigin[:, BG_D2P:BG_D2P + 78] = d2p
        bigin[:, BG_IDENT:BG_IDENT + 128] = np.eye(128, dtype=np.float32)
        bigin[:, BG_AUX:BG_AUX + 10] = aux
        bigin[0, BG_AUX4:BG_AUX4 + 8] = aux4
        atpb = np.zeros((128, AB_W), np.float32)
        atpb[:, AB_ATP:AB_ATP + 192] = atp
        atpb[:, AB_WCT:AB_WCT + 96] = wct
        maps.append(dict(lrb=lrb, bigin=bigin,
                         atpb=atpb.astype(ml_dtypes.bfloat16)))
    return maps


_NC_CACHE = None


def _get_nc():
    global _NC_CACHE
    if _NC_CACHE is None:
        _NC_CACHE = _build()
    return _NC_CACHE


def kernel(X, Y, bandwidths, perms):
    nc = _get_nc()
    in_maps = _host_prep(X, Y, bandwidths, perms)
    res = bass_utils.run_bass_kernel_spmd(nc, in_maps, list(range(NC)))
    full = np.zeros((4, 1 + NPER), np.float32)
    full[:, 0] = res.results[0]["out"][:, 0]
    for cid in range(NC):
        full[:, 1 + cid * PPC:1 + (cid + 1) * PPC] = \
            res.results[cid]["out"][:, 1:]
    return full


# revision 27
# speedup vs baseline: 1.0948x; 1.0089x over previous
"""Trainium2 Bass kernel for the 4-kernel MMD permutation test (nn_DUAL_78237124264373).

Math (per core, 25 of the 200 permutations; everything else replicated):
  Z = [X; Y] (768 x 64). The full squared-distance matrix lands in PSUM as a
  single rank-66 matmul d2 = L^T R with L = [Zt; 1; sq], R = [-2 Zt; sq; 1]
  (sq folded in on the host), so the gaussian kernels exp straight out of
  PSUM with per-kernel scalar scale/bias only. The laplacian kernels go
  through a DVE clamp -> one wide Scalar sqrt -> exp.
  With a_p the X-half indicator of permutation p, every U_b entry reduces to
     U_b = kap*(q0 - arow0) + W_corr @ e_k + (2/c2)*t + C_k
  where q0 = a K0 a, arow0 = a K0 1 come from M0 = A_aug K0 (col-tiled so
  kernel k / perm i stats live at partition 32k+i), e = the zeroed stripe
  K0[j, 384+j], and t is the per-permutation paired-sample sum. Pair and
  stripe squared distances are host-precomputed (d2p, 40KB) and share the
  device sqrt/exp path, so no pair-row gather DMA is needed. The U column
  falls out of q0/arow at the two augmented indicator rows (25 = all-X,
  26 = all-Y).

Final assembly transposes the per-partition stats onto rows with one PE
transpose matmul (identity rhs) and emits the whole [4, 26] result in one
DMA.
"""

import sys

import numpy as np

if "/opt/trn_rl_repo" not in sys.path:
    sys.path.insert(0, "/opt/trn_rl_repo")

import concourse.bacc as bacc
import concourse.bass as bass
import concourse.mybir as mybir
import concourse.tile as tile
from concourse import bass_utils

N = 384
NM = 768
D = 64
NPER = 200
NC = 8
PPC = NPER // NC  # 25
NP6 = 6 * NM      # 4608
WK = NP6 + 78     # 4686 = kernel-matrix cols + pair/stripe cols
C1 = float(N * (N - 1))
C2 = float(N * N)
KAP = np.float32(2.0 / C1 + 2.0 / C2)
CB1 = np.float32(1.0 / C1 + 2.0 / C2)
CB2 = np.float32(1.0 / C1)
TCO = np.float32(2.0 / C2)
IC1 = np.float32(1.0 / C1)
IC2 = np.float32(1.0 / C2)
KERNELS = ("gaussian", "laplacian", "gaussian", "laplacian")

F32 = mybir.dt.float32
F32R = mybir.dt.float32r
BF16 = mybir.dt.bfloat16
AF = mybir.ActivationFunctionType
ALU = mybir.AluOpType

# bigin f32 column layout
BG_ASTK = 0            # [128, 768] A_aug stacked per kernel group
BG_D2P = 768           # [128, 78] pair + stripe squared distances
BG_IDENT = 846         # [128, 128] identity (PE transpose rhs)
BG_AUX = 974           # [128, 10] per-kernel act scale/bias + eps
BG_AUX4 = 984          # [1, 8] row-0 diag constants
BG_W = 992
# atpb bf16 column layout
AB_ATP = 0             # [128, 192] A_aug^T chunks (32-padded)
AB_WCT = 192           # [128, 96] W_corr^T chunks (row 25 = ones -> se)
AB_W = 288


def _build():
    nc = bacc.Bacc("TRN2", target_bir_lowering=False, debug=False)
    with tile.TileContext(nc) as tc:
        with tc.tile_pool(name="dram", bufs=1, space="DRAM") as dram, \
             tc.tile_pool(name="io", bufs=1) as io, \
             tc.tile_pool(name="big", bufs=1) as big, \
             tc.tile_pool(name="kpool", bufs=4) as kpool, \
             tc.tile_pool(name="scr", bufs=1) as scr, \
             tc.tile_pool(name="sml", bufs=1) as sml:

            lrb_d = dram.tile([70, 2 * NM], BF16, kind="ExternalInput",
                              name="lrb", uniquify=False)
            bigin_d = dram.tile([128, BG_W], F32, kind="ExternalInput",
                                name="bigin", uniquify=False)
            atpb_d = dram.tile([128, AB_W], BF16, kind="ExternalInput",
                               name="atpb", uniquify=False)
            out_d = dram.tile([4, 1 + PPC], F32, kind="ExternalOutput",
                              name="out", uniquify=False)

            # ---- phase 0: input DMAs (L|R first: it gates the PE) ----
            lrb = io.tile([70, 2 * NM], BF16, name="lrb_sb")
            nc.sync.dma_start(out=lrb[:, NM:2 * NM], in_=lrb_d[:, NM:2 * NM])
            nc.sync.dma_start(out=lrb[:, 0:128], in_=lrb_d[:, 0:128])
            nc.sync.dma_start(out=lrb[:, 128:NM], in_=lrb_d[:, 128:NM])
            bigin = io.tile([128, BG_W], F32, name="bigin_sb")
            nc.gpsimd.dma_start(out=bigin[:, BG_AUX:BG_W],
                                in_=bigin_d[:, BG_AUX:BG_W])
            atpb = io.tile([128, AB_W], BF16, name="atpb_sb")
            nc.sync.dma_start(out=atpb[:], in_=atpb_d[:])
            nc.sync.dma_start(out=bigin[:, 0:BG_AUX],
                              in_=bigin_d[:, 0:BG_AUX])

            astk = bigin[:, BG_ASTK:BG_ASTK + NM]
            d2pv = bigin[:, BG_D2P:BG_D2P + 78]
            identf = bigin[:, BG_IDENT:BG_IDENT + 128]
            aux = bigin[:, BG_AUX:BG_AUX + 10]
            aux4 = bigin[0:1, BG_AUX4:BG_AUX4 + 8]
            atp = atpb[:, AB_ATP:AB_ATP + 192]
            wct = atpb[:, AB_WCT:AB_WCT + 96]

            onesb = io.tile([128, 1], BF16, name="onesb_sb")
            nc.vector.memset(onesb[:], 1.0)
            onesf = io.tile([128, 1], F32, name="onesf_sb")
            nc.vector.memset(onesf[:], 1.0)
            fsrc = io.tile([128, 512], BF16, name="fsrc_sb")
            nc.vector.memset(fsrc[:], 0.0)

            # d2sb cols 0:4608 = clamped d2 row-tiles; 4608:4686 = host pair
            # d2, so ONE wide sqrt covers both.
            d2sb = big.tile([128, WK], F32, name="d2sb")
            dist = big.tile([128, WK], F32, name="dist_sb")
            kts = [kpool.tile([128, WK], BF16, name=f"kt{k}", tag="kt")
                   for k in range(4)]

            # Warm-up: loads the Exp act table while DMAs are in flight.
            warm = sml.tile([128, 1], F32, name="warm")
            nc.scalar.activation(warm[:], onesf[:], AF.Exp, scale=1.0,
                                 bias=onesf[:, 0:1])

            with tc.tile_pool(name="psM", bufs=1, space="PSUM") as psM:
                ps_m = psM.tile([128, NM], F32, name="ps_m")

                def m0(k, c, start, stop):
                    pr = slice(32 * k, 32 * k + 32)
                    lhsA = atp[:, 32 * c:32 * (c + 1)]
                    for s in (slice(0, 512), slice(512, NM)):
                        nc.tensor.matmul(ps_m[pr, s], lhsA,
                                         kts[k][:, NM * c + s.start:
                                                NM * c + s.stop],
                                         start=start, stop=stop,
                                         tile_position=(0, 32 * k),
                                         skip_group_check=True)

                # Warm the PE p-state while the input DMAs are in flight;
                # ps_m is re-zeroed by every M0 group's start flag.
                for _ in range(2):
                    nc.tensor.matmul(ps_m[:, 0:512], fsrc[:, 0:128],
                                     fsrc[:], start=True, stop=True,
                                     skip_group_check=True)

                with tc.tile_pool(name="psA", bufs=3, space="PSUM") as psA:
                    # ---- phase 1: d2 row-tiles on the PE (f32r), gaussian
                    # exps straight from PSUM, DVE clamp into d2sb ----
                    for r in range(6):
                        ps_d2 = psA.tile([128, NM], F32, name=f"ps_d2_{r}",
                                         tag="d2")
                        # K=70 bf16: rows 0:64 Zt / -2Zt, rows 64:70 carry
                        # the ones/sq rank-2 terms, sq split hi+mid+lo and
                        # derived from the bf16 z so the diagonal cancels.
                        lhsZ = lrb[:, 128 * r:128 * (r + 1)]
                        for s in (slice(0, 512), slice(512, NM)):
                            nc.tensor.matmul(ps_d2[:, s], lhsZ,
                                             lrb[:, NM + s.start:NM + s.stop],
                                             start=True, stop=True)
                        sl = slice(NM * r, NM * (r + 1))
                        for k in (0, 2):
                            nc.scalar.activation(
                                kts[k][:, sl], ps_d2[:], AF.Exp,
                                scale=aux[:, 2 * k:2 * k + 1],
                                bias=aux[:, 2 * k + 1:2 * k + 2])
                        nc.vector.tensor_scalar(
                            out=d2sb[:, sl], in0=ps_d2[:],
                            scalar1=0.0, scalar2=0.0,
                            op0=ALU.max, op1=ALU.add)
                        if r == 0:
                            # pair d2 into the tail cols (host data, early)
                            nc.vector.tensor_copy(d2sb[:, NP6:WK], d2pv)
                        if r in (2, 4):
                            # pair exps absorb the tile-production gaps
                            k = 0 if r == 2 else 2
                            nc.scalar.activation(
                                kts[k][:, NP6:WK], d2sb[:, NP6:WK], AF.Exp,
                                scale=aux[:, 2 * k:2 * k + 1],
                                bias=aux[:, 2 * k + 1:2 * k + 2])

                    # ---- phase 2: gaussian M0 accumulation (col-tiled;
                    # kernel k's stats land at partitions 32k+i) ----
                    for c in range(6):
                        for k in (0, 2):
                            m0(k, c, start=(c == 0), stop=(c == 5))

                # ---- phase 3: dist = sqrt(d2 + 1e-12) incl pair cols,
                # laplacian exps (half-tiles, k-interleaved), lap M0 ----
                nc.scalar.activation(dist[:], d2sb[:], AF.Sqrt,
                                     scale=1.0, bias=aux[:, 8:9])

                with tc.tile_pool(name="psS", bufs=1, space="PSUM") as psS:
                    ps_tr = psS.tile([1, 384], F32, name="ps_tr")
                    ps_corr = psS.tile([128, 1], F32, name="ps_corr")
                    ps_T = psS.tile([1, 512], F32, name="ps_T")
                    ps_fill = psS.tile([128, 512], F32, name="ps_fill")

                    stk = sml.tile([128, 4], F32, name="stk")
                    sB = scr.tile([128, NM], F32, name="sB", tag="sB")
                    sP = scr.tile([128, NM], F32, name="sP", tag="sP")
                    trow = sml.tile([1, 4 * PPC], F32, name="trow")

                    def fill(n):
                        # keep the PE p-state hot across exp-wait gaps
                        for _ in range(n):
                            nc.tensor.matmul(ps_fill[:, 0:512],
                                             fsrc[:, 0:128], fsrc[:],
                                             start=True, stop=True,
                                             skip_group_check=True)

                    def corr_k(k):
                        for c in range(3):
                            nc.tensor.matmul(
                                ps_corr[32 * k:32 * k + 32, 0:1],
                                wct[:, 32 * c:32 * (c + 1)],
                                kts[k][:, NP6 + 75 + c:NP6 + 76 + c],
                                start=(c == 0), stop=(c == 2),
                                tile_position=(0, 32 * k),
                                skip_group_check=True)

                    def trow_mm(k):
                        nc.tensor.matmul(ps_tr[0:1, 96 * k:96 * k + 75],
                                         onesb[:, 0:1],
                                         kts[k][:, NP6:NP6 + 75],
                                         start=True, stop=True)

                    def trow_red(k):
                        nc.vector.tensor_reduce(
                            trow[0:1, PPC * k:PPC * (k + 1)],
                            ps_tr[0:1, 96 * k:96 * k + 75].rearrange(
                                "o (p t) -> o p t", t=3),
                            axis=mybir.AxisListType.X, op=ALU.add)

                    # gaussian t-term + corrections (early; PE idles anyway)
                    for k in (0, 2):
                        trow_mm(k)
                        corr_k(k)
                        trow_red(k)

                    # laplacian exps in half-tiles so M0 can chase them
                    H = 3 * NM  # 2304
                    for k in (1, 3):
                        nc.scalar.activation(
                            kts[k][:, 0:H], dist[:, 0:H], AF.Exp,
                            scale=aux[:, 2 * k:2 * k + 1],
                            bias=aux[:, 9:10])
                    nc.scalar.activation(
                        kts[1][:, H:WK], dist[:, H:WK], AF.Exp,
                        scale=aux[:, 2:3], bias=aux[:, 9:10])
                    for sl in (slice(H, H + NM), slice(H + NM, H + 2 * NM),
                               slice(H + 2 * NM, WK)):
                        nc.scalar.activation(
                            kts[3][:, sl], dist[:, sl], AF.Exp,
                            scale=aux[:, 6:7], bias=aux[:, 9:10])
                    fill(24)
                    for k in (1, 3):
                        for c in range(3):
                            m0(k, c, start=(c == 0), stop=False)
                    fill(6)
                    for c in range(3, 6):
                        m0(1, c, start=False, stop=(c == 5))
                    trow_mm(1)
                    corr_k(1)
                    trow_red(1)
                    fill(2)
                    m0(3, 3, start=False, stop=False)
                    m0(3, 4, start=False, stop=False)
                    m0(3, 5, start=False, stop=True)
                    trow_mm(3)
                    corr_k(3)
                    trow_red(3)
                    nc.vector.tensor_scalar_mul(trow[:], trow[:], float(TCO))
                    nc.vector.tensor_copy(stk[:, 3:4], ps_corr[:])
                    nc.tensor.matmul(ps_T[0:1, 384:512], stk[:, 3:4], identf,
                                     is_transpose=True, start=True, stop=True)

                    # ---- combined row stats: q0 on DVE, arow on Scalar ----
                    nc.vector.tensor_tensor(out=sB[:], in0=ps_m[:],
                                            in1=astk[:], op=ALU.mult)
                    nc.vector.tensor_reduce(stk[:, 1:2], sB[:],
                                            axis=mybir.AxisListType.X,
                                            op=ALU.add)
                    nc.scalar.activation(sP[:], ps_m[:], AF.Copy,
                                         scale=float(KAP), bias=0.0,
                                         accum_out=stk[:, 2:3])
                    for j in (1, 2):
                        nc.tensor.matmul(ps_T[0:1, 128 * j:128 * (j + 1)],
                                         stk[:, j:j + 1], identf,
                                         is_transpose=True,
                                         start=True, stop=True)
                    u0 = sml.tile([128, 1], F32, name="u0")
                    nc.vector.tensor_tensor(out=u0[:], in0=stk[:, 1:2],
                                            in1=stk[:, 2:3], op=ALU.subtract)
                    nc.vector.tensor_tensor(out=stk[:, 0:1], in0=u0[:],
                                            in1=ps_corr[:], op=ALU.add)
                    nc.tensor.matmul(ps_T[0:1, 0:128], stk[:, 0:1], identf,
                                     is_transpose=True, start=True, stop=True)
                    frow = sml.tile([1, 384], F32, name="frow")
                    nc.vector.tensor_copy(frow[:], ps_T[0:1, 128:512])

                    # ---- U stats on transposed rows (all kap-scaled) ----
                    def fr(base, step=32, count=4):
                        ap = frow[0:1, base:base + 1]
                        return bass.AP(ap.tensor, ap.offset,
                                       [ap.ap[0], [step, count]])

                    IC1K = float(IC1 / KAP)
                    XXv = fr(25)         # kap * 1_X K0 1_X
                    YYv = fr(26)         # kap * 1_Y K0 1_Y
                    a25v = fr(128 + 25)  # kap * 1_X K0 1
                    a26v = fr(128 + 26)  # kap * 1_Y K0 1
                    sevv = fr(256 + 25)  # kap * se_k (kap row of W_corr)
                    s12 = sml.tile([1, 4], F32, name="s12")
                    nc.vector.tensor_tensor(out=s12[:], in0=a25v, in1=a26v,
                                            op=ALU.add)
                    nc.vector.tensor_tensor(out=s12[:], in0=s12[:], in1=sevv,
                                            op=ALU.subtract)
                    nc.vector.tensor_tensor(out=s12[:], in0=s12[:],
                                            in1=aux4[0:1, 0:4],
                                            op=ALU.subtract)
                    ck = sml.tile([1, 4], F32, name="ck")
                    nc.vector.tensor_scalar_mul(ck[:], s12[:], IC1K)
                    u1 = sml.tile([1, 4], F32, name="u1")
                    nc.vector.tensor_tensor(out=u1[:], in0=XXv, in1=YYv,
                                            op=ALU.add)
                    nc.vector.tensor_tensor(out=u1[:], in0=u1[:],
                                            in1=aux4[0:1, 0:4],
                                            op=ALU.subtract)
                    nc.vector.tensor_scalar_mul(u1[:], u1[:], IC1K)
                    u2 = sml.tile([1, 4], F32, name="u2")
                    nc.vector.tensor_tensor(out=u2[:], in0=a25v, in1=XXv,
                                            op=ALU.subtract)
                    nc.vector.tensor_tensor(out=u2[:], in0=u2[:], in1=sevv,
                                            op=ALU.subtract)
                    nc.vector.tensor_scalar_mul(u2[:], u2[:],
                                                float(-2.0 * IC2 / KAP))

                    # ---- contiguous [1, 104] result, one DMA ----
                    uball = sml.tile([1, 4 * (1 + PPC)], F32, name="uball")
                    uball0 = uball[0:1, 0:1]
                    uFv = bass.AP(uball0.tensor, uball0.offset,
                                  [uball0.ap[0], [1 + PPC, 4]])
                    nc.vector.tensor_tensor(out=uFv, in0=u1[:], in1=u2[:],
                                            op=ALU.add)
                    ubv = bass.AP(uball0.tensor, uball0.offset + 1,
                                  [uball0.ap[0], [1 + PPC, 4], [1, PPC]])
                    ub_src = ps_T[0:1, 0:128].rearrange(
                        "o (k p) -> o k p", p=32)
                    ckap = ck[0:1, 0:1]
                    ck_b = bass.AP(ckap.tensor, ckap.offset,
                                   [ckap.ap[0], [1, 4], [0, PPC]])
                    nc.vector.tensor_tensor(out=ubv,
                                            in0=ub_src[0:1, :, 0:PPC],
                                            in1=ck_b, op=ALU.add)
                    trow_v = trow[0:1, :].rearrange("o (k p) -> o k p", p=PPC)
                    nc.vector.tensor_tensor(out=ubv, in0=ubv, in1=trow_v,
                                            op=ALU.add)
                    nc.gpsimd.dma_start(
                        out=out_d[:, :],
                        in_=uball[0:1, :].rearrange("o (k w) -> o k w",
                                                    w=1 + PPC))

    nc.compile()
    return nc


def _host_prep(X, Y, bandwidths, perms):
    X = np.ascontiguousarray(X, np.float32)
    Y = np.ascontiguousarray(Y, np.float32)
    perms = np.ascontiguousarray(perms, np.int32)
    import ml_dtypes

    Zf = np.concatenate([X, Y], 0)                  # [768, 64]
    Ztb = Zf.T.astype(ml_dtypes.bfloat16)           # [64, 768] device dtype
    Zb64 = Ztb.astype(np.float64).T                 # bf16-rounded z, f64
    sqb = np.einsum("ij,ij->i", Zb64, Zb64)         # exact device row norms
    Z64 = Zf.astype(np.float64)
    sq64 = np.einsum("ij,ij->i", Z64, Z64)
    sqhi = sqb.astype(np.float32).astype(ml_dtypes.bfloat16)
    r1 = sqb - sqhi.astype(np.float64)
    sqmid = r1.astype(np.float32).astype(ml_dtypes.bfloat16)
    r2 = r1 - sqmid.astype(np.float64)
    sqlo = r2.astype(np.float32).astype(ml_dtypes.bfloat16)
    lrb = np.zeros((70, 2 * NM), ml_dtypes.bfloat16)
    lrb[0:D, 0:NM] = Ztb
    lrb[0:D, NM:] = (-2.0 * Ztb.astype(np.float32)).astype(ml_dtypes.bfloat16)
    lrb[D:D + 3, 0:NM] = 1.0
    lrb[D + 3, 0:NM] = sqhi
    lrb[D + 4, 0:NM] = sqmid
    lrb[D + 5, 0:NM] = sqlo
    lrb[D, NM:] = sqhi
    lrb[D + 1, NM:] = sqmid
    lrb[D + 2, NM:] = sqlo
    lrb[D + 3:D + 6, NM:] = 1.0

    b = np.asarray(bandwidths, np.float64)
    gs = (-1.0 / (b * b)).astype(np.float32)
    ls = (-1.0 / b).astype(np.float32)
    aux = np.zeros((128, 10), np.float32)
    BETA = 0.01
    aux[:, 8] = BETA
    d0c = np.zeros(4, np.float64)
    for k, kern in enumerate(KERNELS):
        if kern == "gaussian":
            aux[:, 2 * k] = gs[k]
            aux[:, 2 * k + 1] = (gs[k].astype(np.float64) * 1e-12
                                 ).astype(np.float32)
            d0c[k] = np.exp(-1e-12 / (b[k] * b[k]))
        else:
            aux[:, 2 * k] = ls[k]
            d0c[k] = np.exp(-np.sqrt(BETA) / b[k])
    aux4 = np.zeros(8, np.float32)
    aux4[0:4] = (768.0 * d0c * float(KAP)).astype(np.float32)

    maps = []
    for cid in range(NC):
        pm = perms[cid * PPC:(cid + 1) * PPC]
        A = np.zeros((27, NM), np.float32)
        A[np.arange(PPC)[:, None], pm[:, :N]] = 1
        A[25, :N] = 1
        A[26, N:] = 1
        astk = np.zeros((128, NM), np.float32)
        for k in range(4):
            astk[32 * k:32 * k + 27] = A * KAP
        atp = np.zeros((128, 6 * 32), np.float32)
        for c in range(6):
            atp[:, 32 * c:32 * c + 27] = A[:, 128 * c:128 * (c + 1)].T
        A1 = A[:PPC, :N]
        A2 = A[:PPC, N:]
        Wc = (-KAP * (A1 * A2) + CB1 * A1 + CB2 * A2).astype(np.float32)
        wct = np.zeros((128, 3 * 32), np.float32)
        for c in range(3):
            wct[:, 32 * c:32 * c + PPC] = Wc[:, 128 * c:128 * (c + 1)].T
            wct[:, 32 * c + 25] = KAP  # kap*se_k extraction row

        # pair + stripe squared distances, f64 on host
        pX = pm[:, :N].astype(np.int64).ravel()
        pY = pm[:, N:].astype(np.int64).ravel()
        j = np.arange(N)
        pa = np.concatenate([pX, j])
        pb = np.concatenate([pY, N + j])
        d2pair = (sq64[pa] + sq64[pb]
                  - 2.0 * np.einsum("ij,ij->i", Z64[pa], Z64[pb]))
        d2pair = np.maximum(d2pair, 0.0) + 1e-12
        stripe = pY == pX + N
        d2pair[:N * PPC][stripe] = 1e12  # zeroed-stripe pairs: f_k -> 0
        d2p = d2pair.astype(np.float32).reshape(78, 128).T  # [128, 78]

        bigin = np.zeros((128, BG_W), np.float32)
        bigin[:, BG_ASTK:BG_ASTK + NM] = astk
        bigin[:, BG_D2P:BG_D2P + 78] = d2p
        bigin[:, BG_IDENT:BG_IDENT + 128] = np.eye(128, dtype=np.float32)
        bigin[:, BG_AUX:BG_AUX + 10] = aux
        bigin[0, BG_AUX4:BG_AUX4 + 8] = aux4
        atpb = np.zeros((128, AB_W), np.float32)
        atpb[:, AB_ATP:AB_ATP + 192] = atp
        atpb[:, AB_WCT:AB_WCT + 96] = wct
        maps.append(dict(lrb=lrb, bigin=bigin,
                         atpb=atpb.astype(ml_dtypes.bfloat16)))
    return maps


_NC_CACHE = None


def _get_nc():
    global _NC_CACHE
    if _NC_CACHE is None:
        _NC_CACHE = _build()
    return _NC_CACHE


def kernel(X, Y, bandwidths, perms):
    nc = _get_nc()
    in_maps = _host_prep(X, Y, bandwidths, perms)
    res = bass_utils.run_bass_kernel_spmd(nc, in_maps, list(range(NC)))
    full = np.zeros((4, 1 + NPER), np.float32)
    full[:, 0] = res.results[0]["out"][:, 0]
    for cid in range(NC):
        full[:, 1 + cid * PPC:1 + (cid + 1) * PPC] = \
            res.results[cid]["out"][:, 1:]
    return full
